# revision 64
# baseline (speedup 1.0000x reference)
"""3-layer GAT on 8 Trainium2 NeuronCores (Bass/Tile).

Sharding: 2D graph partition. Pair q = cores {2q, 2q+1} aggregates the dst
nodes of strips [q*6250,(q+1)*6250) and [25000+q*6250, 25000+(q+1)*6250);
even cores take edges with src < 25000, odd cores the rest. Node ownership:
core 2k owns rows [k*6250,(k+1)*6250), core 2k+1 owns [25000+k*6250, ...).

Per layer: each core projects its own rows (feat|el|er via an augmented
weight matrix) into a local gather table, then runs the edge phase in two
passes: pass A covers edges whose source is one of the core's own rows and
gathers from the local table while the quad AllGather of the full src-half
table is still in flight; pass B covers the remaining edges and gathers
from the AllGathered table. Edges are packed into 128-wide tiles grouped
by pairs of 128-dst blocks (a tile may straddle the two blocks; the
host-precomputed one-hot masks select membership). Per-edge er comes from
a transposed one-hot matmul against SBUF-resident per-block er rows (no
second gather). Messages are accumulated per dst block by one-hot-mask
matmuls into PSUM; pass B adds pass A's partial sums back in.

The dst pair-groups are processed in 4 segments; the pairwise
ReduceScatter of partial sums is split into 4 matching pieces (the
partial-row permutation interleaves [A-range | B-range] per piece so each
piece is contiguous), each fired as soon as its blocks complete mid-pass-B.
Post-processing (divide/bias/ELU, head-mean on the last layer) and the
next layer's projection are streamed per own-block range behind each RS
piece, so almost the whole layer boundary hides under pass B. The er
AllGather is split per range the same way (er_tab shares the partial-row
permutation, keeping each piece contiguous); mask loads for the first
chunks of each pass are prefetched during the previous pass. Gather calls
carry trailing -1 indices so the Q7 descriptor generator truncates padded
tail slots.
"""

import numpy as np
import ml_dtypes

N = 50000
E = 800000
F = 128                  # input feats and hidden width (4 heads x 32)
H = 4
D = 32
NEG = 0.2
NCORE = 8
NPC = 6250               # nodes owned per core
OWN = 6272               # 49*128, padded own rows
OWNBLK = 49
PAIR = 12544             # 98*128 dst slots per pair
NBLK = 98
NGRP = 49                # pair-groups of 2 blocks
HALF = 25088             # 4*OWN rows per src-half table
TROWS = 25216            # HALF + 128 (dummy row at HALF)
DUMMY = HALF
TCOLS = 256              # bf16 cols: feat(128) | el(4) | pad
PCOLS = 132              # packed AllGather row: feat(128) | el(4)
CHUNK = 24               # max tiles per dma_gather call
GROUP = 8                # tiles per vector-op batch
EPS = 1e-30

# Own-block ranges: post/proj are streamed per range, each behind its own
# ReduceScatter piece. The last range is small so the layer-boundary chain
# (last RS piece -> post -> proj -> table write) is short.
OWN_RANGES = [(0, 22), (22, 34), (34, 45), (45, 49)]

# pass-B pair-group order: 4 segments, segment r completes the blocks of
# OWN_RANGES[r] (A side: blocks lo..hi-1, B side: 49+lo..49+hi-1), so each
# ReduceScatter piece fires as early as possible. Straddle groups whose
# later-range block completes early are harmless (counters are per block).
GORDER_B = (list(range(0, 11)) + list(range(24, 36)) +    # seg 1
            list(range(11, 17)) + list(range(36, 42)) +   # seg 2
            list(range(17, 23)) + list(range(42, 47)) +   # seg 3
            [23, 47, 48])                                 # seg 4

# pass-A pair-group order: sorted by the er-AllGather piece each group
# needs (max over its two blocks), so groups needing the late pieces sit at
# the end of pass A and never stall the in-order engine queues.
GORDER_A = (list(range(0, 11)) + list(range(25, 35)) +    # er range 1
            list(range(11, 17)) + list(range(35, 41)) +   # er range 2
            list(range(17, 22)) + list(range(41, 47)) +   # er range 3
            [22, 23, 24, 47, 48])                         # er range 4

# block -> partial-row-block permutation: [A1 B1 | A2 B2 | A3 B3 | A4 B4]
# where Ar/Br are the A/B-side blocks of OWN_RANGES[r]. The same layout is
# used for er_tab so the split pair-AllGather outputs stay contiguous.
ROW_OF = np.empty(NBLK, np.int64)
RNG_OF = np.empty(NBLK, np.int64)    # block -> range index
LROW_OF = np.empty(NBLK, np.int64)   # block -> row-block within its range
RS_PLAN = []             # (blockset, p_lo, p_hi, o_lo, o_hi) per range
_base = 0
for _ri, (_lo, _hi) in enumerate(OWN_RANGES):
    _sz = _hi - _lo
    ROW_OF[_lo:_hi] = _base + np.arange(_sz)
    ROW_OF[49 + _lo:49 + _hi] = _base + _sz + np.arange(_sz)
    RNG_OF[_lo:_hi] = _ri
    RNG_OF[49 + _lo:49 + _hi] = _ri
    LROW_OF[_lo:_hi] = np.arange(_sz)
    LROW_OF[49 + _lo:49 + _hi] = _sz + np.arange(_sz)
    _blocks = frozenset(range(_lo, _hi)) | frozenset(range(49 + _lo, 49 + _hi))
    RS_PLAN.append((_blocks, _base * 128, (_base + 2 * _sz) * 128,
                    _lo * 128, _hi * 128))
    _base += 2 * _sz
assert _base == NBLK

# Range-major layout of the quad-AllGathered src-half table: range r holds
# its 4 ranks' own-row stripes contiguously, so the table AllGather (and
# the local 264B->512B re-stripe) splits into 4 contiguous pieces.
T_PBASE = []
_tb = 0
for _lo, _hi in OWN_RANGES:
    T_PBASE.append(_tb)
    _tb += 4 * (_hi - _lo) * 128
assert _tb == HALF
# own-local row -> range-major table row offset (within one rank's stripe
# the rows keep own-local order; rank k of range r starts at
# T_PBASE[r] + k * sz_r * 128).
_OROW_RANGE = np.empty(OWN, np.int64)   # own row -> range idx
_OROW_LOCAL = np.empty(OWN, np.int64)   # own row -> row within range stripe
for _ri, (_lo, _hi) in enumerate(OWN_RANGES):
    _OROW_RANGE[_lo * 128:_hi * 128] = _ri
    _OROW_LOCAL[_lo * 128:_hi * 128] = np.arange((_hi - _lo) * 128)

_cache = {}


def _schedule(cnt, gorder):
    """Core-uniform tile/mask schedule for one pass.

    cnt: [NCORE, NBLK] per-core per-block edge counts.
    Returns dict with T, nmask, chunks, tiles (per tile: list of
    (mslot, block, er_first, er_last, sc_first, sc_last)).
    """
    GORDER = gorder
    n0 = cnt[:, 0::2]                      # [NCORE, NGRP]
    n1 = cnt[:, 1::2]
    TP = np.maximum(1, np.ceil((n0 + n1).max(axis=0) / 128).astype(np.int64))

    base_tile = {}
    acc = 0
    for g in GORDER:
        base_tile[g] = acc
        acc += int(TP[g])
    T = acc

    tiles = []            # per tile: list of [mslot, block]
    tile_group = []
    # which (tile-in-group, block-parity) pairs are needed on any core; ensure
    # every block gets at least one occurrence (tile 0 fallback)
    need = {}
    for g in GORDER:
        for i in range(int(TP[g])):
            need[(g, i, 0)] = bool((n0[:, g] > 128 * i).any())
            need[(g, i, 1)] = bool(
                ((n0[:, g] < 128 * (i + 1)) &
                 (n0[:, g] + n1[:, g] > 128 * i)).any())
        if not any(need[(g, i, 0)] for i in range(int(TP[g]))):
            need[(g, 0, 0)] = True
        if not any(need[(g, i, 1)] for i in range(int(TP[g]))):
            need[(g, 0, 1)] = True
    mslot = 0
    for g in GORDER:
        for i in range(int(TP[g])):
            ml = []
            if need[(g, i, 0)]:
                ml.append([mslot, 2 * g])
                mslot += 1
            if need[(g, i, 1)]:
                ml.append([mslot, 2 * g + 1])
                mslot += 1
            assert ml
            tiles.append(ml)
            tile_group.append(g)
    nmask = mslot

    # per-block first/last occurrence
    occ = {}
    for ti, ml in enumerate(tiles):
        for m in ml:
            occ.setdefault(m[1], []).append((ti, m[0]))
    first = {b: o[0] for b, o in occ.items()}
    last = {b: o[-1] for b, o in occ.items()}
    sched_tiles = []
    for ti, ml in enumerate(tiles):
        entry = []
        for k, (ms, b) in enumerate(ml):
            entry.append((ms, b,
                          k == 0, k == len(ml) - 1,
                          first[b] == (ti, ms), last[b] == (ti, ms)))
        sched_tiles.append(entry)

    # chunks aligned to pair-group boundaries, up to CHUNK tiles
    chunks = []
    t0 = 0
    ti = 0
    for g in GORDER:
        ti += int(TP[g])
        nxt = None
        gi = GORDER.index(g)
        if gi + 1 < len(GORDER):
            nxt = int(TP[GORDER[gi + 1]])
        if nxt is None or ti - t0 + nxt > CHUNK:
            m0 = min(m[0] for m in sched_tiles[t0]) if sched_tiles[t0] else 0
            mend = max(m[0] for m in sched_tiles[ti - 1]) + 1
            chunks.append((t0, ti - t0, m0, mend - m0))
            t0 = ti
    assert t0 == T
    return dict(T=T, nmask=nmask, chunks=chunks, tiles=sched_tiles,
                tile_group=tile_group, base_tile=base_tile, TP=TP,
                gorder=gorder)


def _wrap16(a):
    # value i of each 128-group at [i%16, i//16], replicated per 16 rows
    t = a.reshape(-1, 128)                     # [T, 128]
    w = t.reshape(t.shape[0], 8, 16)           # [T, 8, 16]
    w = w.transpose(2, 0, 1).reshape(16, -1)   # [16, T*8]
    return np.tile(w, (8, 1)).astype(np.int16)  # [128, T*8]


def _core_pass_arrays(sched, rloc_e, rows_e, pad_row, trunc_from_chunk):
    """Build idx + mask streams for one (core, pass).

    rloc_e: pair-local dst row per edge; rows_e: gather-table row per edge.
    Chunks with index >= trunc_from_chunk get their trailing padded idx
    slots set to -1 (the Q7 truncates them); earlier chunks keep the dummy
    row so first-touch SBUF tiles never expose uninitialized data.
    """
    T, nmask = sched["T"], sched["nmask"]
    base_tile = sched["base_tile"]
    GORDER = sched["gorder"]
    # group rank of each edge
    grank_of = np.empty(NGRP, np.int64)
    for r, g in enumerate(GORDER):
        grank_of[g] = r
    pg = rloc_e // 256
    gr = grank_of[pg]
    order = np.lexsort((rloc_e, gr))
    rloc_s = rloc_e[order]
    rows_s = rows_e[order]
    gr_s = gr[order]
    # position within group
    starts = np.searchsorted(gr_s, np.arange(len(GORDER)))
    pos_in_group = np.arange(len(gr_s)) - starts[gr_s]
    base128 = np.array([base_tile[GORDER[r]] * 128
                        for r in range(len(GORDER))], np.int64)
    s_glob = base128[gr_s] + pos_in_group

    idx = np.full(T * 128, pad_row, np.int64)
    idx[s_glob] = rows_s
    real = np.zeros(T * 128, bool)
    real[s_glob] = True
    for ci, (t0, nt, _m0, _nm) in enumerate(sched["chunks"]):
        if ci < trunc_from_chunk:
            continue
        a, b = t0 * 128, (t0 + nt) * 128
        nz = np.flatnonzero(real[a:b])
        last = nz[-1] if len(nz) else -1
        idx[a + last + 1:b] = -1

    # mask slot lookup per (tile, block-parity)
    mslot_of = np.full((T, 2), -1, np.int64)
    for ti, ml in enumerate(sched["tiles"]):
        g = sched["tile_group"][ti]
        for (ms, b, *_fl) in ml:
            mslot_of[ti, b - 2 * g] = ms
    ti_e = s_glob // 128
    e_e = s_glob % 128
    b_e = rloc_s // 128
    s128_e = rloc_s % 128
    par = b_e - 2 * np.array(sched["tile_group"])[ti_e]
    ms_e = mslot_of[ti_e, par]
    assert (ms_e >= 0).all()

    smatTw = np.zeros((128, nmask * 128), ml_dtypes.bfloat16)
    smatTw[s128_e, ms_e * 128 + e_e] = 1
    return _wrap16(idx), smatTw


def _preprocess(src, dst):
    src = np.asarray(src).astype(np.int64)
    dst = np.asarray(dst).astype(np.int64)
    q = np.where(dst < 25000, dst // NPC, (dst - 25000) // NPC)
    s = (src >= 25000).astype(np.int64)
    core_of = 2 * q + s
    rloc = np.where(dst < 25000, dst - q * NPC, OWN + (dst - 25000 - q * NPC))
    ks = np.where(src < 25000, src // NPC, (src - 25000) // NPC)
    olocal = np.where(src < 25000, src - ks * NPC, (src - 25000) - ks * NPC)
    _ri = _OROW_RANGE[olocal]
    _szr = np.array([(hi - lo) * 128 for lo, hi in OWN_RANGES], np.int64)
    tloc = (np.array(T_PBASE, np.int64)[_ri] + ks * _szr[_ri]
            + _OROW_LOCAL[olocal])
    own = ks == q
    ownrow = np.where(src < 25000, src - ks * NPC, src - 25000 - ks * NPC)
    blk = rloc // 128

    cntA = np.zeros((NCORE, NBLK), np.int64)
    cntB = np.zeros((NCORE, NBLK), np.int64)
    for c in range(NCORE):
        m = core_of == c
        cntA[c] = np.bincount(blk[m & own], minlength=NBLK)
        cntB[c] = np.bincount(blk[m & ~own], minlength=NBLK)
    schedA = _schedule(cntA, GORDER_A)
    schedB = _schedule(cntB, GORDER_B)

    cores = []
    for c in range(NCORE):
        m = core_of == c
        mA = m & own
        mB = m & ~own
        idxA, smTA = _core_pass_arrays(schedA, rloc[mA], ownrow[mA],
                                       OWN, 10**9)
        idxB, smTB = _core_pass_arrays(schedB, rloc[mB], tloc[mB],
                                       DUMMY, 10**9)
        cores.append(dict(idxA=idxA, smTA=smTA, idxB=idxB, smTB=smTB))
    return cores, schedA, schedB


def _own_rows(c):
    k = c // 2
    if c % 2 == 0:
        return k * NPC, (k + 1) * NPC
    return 25000 + k * NPC, 25000 + (k + 1) * NPC


def _augment(W, al, ar):
    dout = W.shape[1] // H
    Wal = np.stack([W[:, h * dout:(h + 1) * dout] @ al[h] for h in range(H)], 1)
    War = np.stack([W[:, h * dout:(h + 1) * dout] @ ar[h] for h in range(H)], 1)
    return np.concatenate([W, Wal, War], 1).astype(np.float32)  # [128, 136]


def _build(schedA, schedB, consts, no_cc=False):
    import concourse.bass as bass
    import concourse.bacc as bacc
    import concourse.tile as tile
    from concourse import mybir
    from concourse.library_config import mlp

    f32 = mybir.dt.float32
    bf16 = mybir.dt.bfloat16
    i16 = mybir.dt.int16
    AF = mybir.ActivationFunctionType
    OP = mybir.AluOpType

    TA, TB = schedA["T"], schedB["T"]
    NMA, NMB = schedA["nmask"], schedB["nmask"]
    NM_MAX = max(max(nm for (_, _, _, nm) in schedA["chunks"]),
                 max(nm for (_, _, _, nm) in schedB["chunks"]))

    nc = bacc.Bacc(num_devices=NCORE)
    xT_in = nc.declare_dram_parameter("xT", [128, OWN], bf16, isOutput=False)
    idxA_in = nc.declare_dram_parameter("idxA", [128, TA * 8], i16,
                                        isOutput=False)
    idxB_in = nc.declare_dram_parameter("idxB", [128, TB * 8], i16,
                                        isOutput=False)
    smTA_in = nc.declare_dram_parameter("smTA", [128, NMA * 128], bf16,
                                        isOutput=False)
    smTB_in = nc.declare_dram_parameter("smTB", [128, NMB * 128], bf16,
                                        isOutput=False)
    y_out = nc.declare_dram_parameter("y", [NPC, D], f32, isOutput=True)

    with tile.TileContext(nc) as tc:
        with tc.tile_pool(name="persist", bufs=1) as pp, \
             tc.tile_pool(name="mk", bufs=3) as mkp, \
             tc.tile_pool(name="dram", bufs=1, space="DRAM") as dp:
            nc.gpsimd.load_library(mlp)

            # ---- persistent SBUF state ----
            idxA_sb = pp.tile([128, TA * 8], i16)
            nc.sync.dma_start(out=idxA_sb[:], in_=idxA_in[:, :])
            idxB_sb = pp.tile([128, TB * 8], i16)
            nc.sync.dma_start(out=idxB_sb[:], in_=idxB_in[:, :])
            hT = pp.tile([128, OWN], bf16)
            hT2 = pp.tile([128, OWN], bf16)

            ident_h = nc.inline_tensor(np.eye(128, dtype=np.float32),
                                       name="ident")
            ident_sb = pp.tile([128, 128], f32)
            nc.sync.dma_start(out=ident_sb[:], in_=ident_h[:, :])
            identb_h = nc.inline_tensor(np.eye(128).astype(ml_dtypes.bfloat16),
                                        name="identb")
            identb_sb = pp.tile([128, 128], bf16)
            nc.sync.dma_start(out=identb_sb[:], in_=identb_h[:, :])

            waug_sb = []
            brep_sb = []
            for li in range(3):
                wh = nc.inline_tensor(consts[f"Waug{li}"], name=f"waug{li}")
                wt = pp.tile([128, 136], bf16, name=f"waug_sb{li}")
                nc.sync.dma_start(out=wt[:], in_=wh[:, :])
                waug_sb.append(wt)
                bh = nc.inline_tensor(consts[f"brep{li}"], name=f"brep{li}")
                bt = pp.tile([128, consts[f"brep{li}"].shape[1]], f32,
                             name=f"brep_sb{li}")
                nc.sync.dma_start(out=bt[:], in_=bh[:, :])
                brep_sb.append(bt)

            dummy_h = nc.inline_tensor(consts["dummyrow"], name="dummyrow")

            # ---- DRAM scratch ----
            # partialA/partial are split per own-block range so pass-B loads
            # of a range only depend on that range's pass-A writes (and the
            # RS pieces only on their own range's writes).
            # The gather table is double buffered by layer parity (the next
            # layer's split AllGather pieces land while the current layer's
            # pass B still gathers from the old table). The AllGather moves
            # packed 264B rows; a local re-stripe expands to the 512B-stride
            # layout the gather needs.
            table = [dp.tile([TROWS, TCOLS], bf16, name=f"table{p}")
                     for p in range(2)]
            table_pack = dp.tile([HALF, PCOLS], bf16)
            er_tab = dp.tile([PAIR, 4], bf16)
            ag_own = dp.tile([OWN + 128, TCOLS], bf16)
            ag_pack = dp.tile([OWN, PCOLS], bf16)
            ag_er = dp.tile([OWN, 4], bf16)
            partialA = [dp.tile([(p_hi - p_lo), 132], f32,
                                name=f"partialA{ri}")
                        for ri, (_b, p_lo, p_hi, _o, _o2) in
                        enumerate(RS_PLAN)]
            partial = [dp.tile([(p_hi - p_lo), 132], f32, name=f"partial{ri}")
                       for ri, (_b, p_lo, p_hi, _o, _o2) in
                       enumerate(RS_PLAN)]
            own_sum = dp.tile([OWN, 132], f32)

            nc.sync.dma_start(out=table[0][DUMMY:DUMMY + 1, :],
                              in_=dummy_h[:, :])
            nc.sync.dma_start(out=table[1][DUMMY:DUMMY + 1, :],
                              in_=dummy_h[:, :])
            nc.sync.dma_start(out=ag_own[OWN:OWN + 1, :], in_=dummy_h[:, :])

            groups_pair = [[2 * k, 2 * k + 1] for k in range(4)]
            groups_quad = [[0, 2, 4, 6], [1, 3, 5, 7]]

            # mask prefetch bookkeeping: (pass_key, ci) -> smT tile. Only
            # smT lives in DRAM; sm is derived on-chip by PE transpose in
            # the front->back slack of the chunk pipeline.
            premask = {}

            def load_masks(key, sched, smT_in, ci):
                if (key, ci) in premask:
                    return premask.pop((key, ci))
                (_t0, _nt, m0, nm) = sched["chunks"][ci]
                smT = mkp.tile([128, NM_MAX * 128], bf16, tag="smT")
                nc.sync.dma_start(
                    out=smT[:, 0:nm * 128],
                    in_=smT_in[:, m0 * 128:(m0 + nm) * 128])
                return smT

            def prefetch_masks(key, sched, smT_in, n=2):
                for ci in range(min(n, len(sched["chunks"]))):
                    premask[(key, ci)] = load_masks(key + "_", sched,
                                                    smT_in, ci)

            # er_sb: per-layer er rows in partial-permuted block order, one
            # tile per range so readers only depend on their own range's
            # split er AllGather. Double buffered by layer parity: the next
            # layer's er loads land while the current layer's edge passes
            # still read the old values.
            er_sb = [[pp.tile([128, (p_hi - p_lo) // 128, 4], bf16,
                              name=f"er_sb{par}_{ri}")
                      for ri, (_b, p_lo, p_hi, _o, _o2) in
                      enumerate(RS_PLAN)]
                     for par in range(2)]

            def load_er_range(li, r):
                (_bl, p_lo, p_hi, _o_lo, _o_hi) = RS_PLAN[r]
                nc.sync.dma_start(
                    out=er_sb[li % 2][r][:],
                    in_=er_tab[p_lo:p_hi, :]
                        .rearrange("(t p) c -> p t c", p=128))

            def proj_range(li, src_hT, r, prange_pool, prj_ps):
                """Project own blocks of OWN_RANGES[r]: write ag_own rows,
                ag_er rows, then fire the split pair-AllGather of er."""
                (o_lo, o_hi) = OWN_RANGES[r]
                for b0 in range(o_lo, o_hi, 8):
                    nb = min(8, o_hi - b0)
                    tabrow = prange_pool.tile([128, 8, TCOLS], bf16,
                                              tag="tabrow")
                    errow = prange_pool.tile([128, 8, 4], bf16, tag="errow")
                    for t in range(nb):
                        pj = prj_ps.tile([128, 136], f32, space="PSUM",
                                         tag="aux")
                        nc.tensor.matmul(
                            pj[:],
                            lhsT=src_hT[:, (b0 + t) * 128:(b0 + t + 1) * 128],
                            rhs=waug_sb[li][:, 0:136], start=True, stop=True)
                        nc.scalar.activation(tabrow[:, t, 0:132],
                                             pj[:, 0:132], AF.Copy)
                        nc.scalar.activation(errow[:, t, :], pj[:, 132:136],
                                             AF.Copy)
                    nc.sync.dma_start(
                        out=ag_own[b0 * 128:(b0 + nb) * 128, :]
                            .rearrange("(t p) c -> p t c", p=128),
                        in_=tabrow[:, 0:nb, :])
                    nc.sync.dma_start(
                        out=ag_pack[b0 * 128:(b0 + nb) * 128, :]
                            .rearrange("(t p) c -> p t c", p=128),
                        in_=tabrow[:, 0:nb, 0:PCOLS])
                    nc.sync.dma_start(
                        out=ag_er[b0 * 128:(b0 + nb) * 128, :]
                            .rearrange("(t p) c -> p t c", p=128),
                        in_=errow[:, 0:nb, :])
                (_bl, p_lo, p_hi, ro_lo, ro_hi) = RS_PLAN[r]
                if no_cc:
                    sz = ro_hi - ro_lo
                    nc.sync.dma_start(out=er_tab[p_lo:p_lo + sz, :],
                                      in_=ag_er[ro_lo:ro_hi, :])
                    nc.sync.dma_start(out=er_tab[p_lo + sz:p_hi, :],
                                      in_=ag_er[ro_lo:ro_hi, :])
                else:
                    nc.gpsimd.collective_compute(
                        "AllGather", mybir.AluOpType.bypass,
                        replica_groups=groups_pair,
                        ins=[ag_er[ro_lo:ro_hi, :]],
                        outs=[er_tab[p_lo:p_hi, :]])
                load_er_range(li, r)

            def table_ag_piece(li, r):
                """Quad-AllGather the packed own rows of range r into the
                contiguous packed table."""
                (o_lo, o_hi) = OWN_RANGES[r]
                sz = (o_hi - o_lo) * 128
                t0 = T_PBASE[r]
                if no_cc:
                    for rep in range(4):
                        nc.sync.dma_start(
                            out=table_pack[t0 + rep * sz:t0 + (rep + 1) * sz,
                                           :],
                            in_=ag_pack[o_lo * 128:o_hi * 128, :])
                else:
                    nc.gpsimd.collective_compute(
                        "AllGather", mybir.AluOpType.bypass,
                        replica_groups=groups_quad,
                        ins=[ag_pack[o_lo * 128:o_hi * 128, :]],
                        outs=[table_pack[t0:t0 + 4 * sz, :]])

            def restripe_piece(li, r):
                # 264B-packed -> 512B-stride expansion; emit only at points
                # where the matching AG piece is already complete, else the
                # in-order sync DMA queue bubbles behind the wait.
                (o_lo, o_hi) = OWN_RANGES[r]
                sz = (o_hi - o_lo) * 128
                t0 = T_PBASE[r]
                nc.sync.dma_start(
                    out=table[li % 2][t0:t0 + 4 * sz, 0:PCOLS],
                    in_=table_pack[t0:t0 + 4 * sz, :])

            def post_range(li, r, prange_pool, prj_ps):
                """Divide/bias/activation for own blocks of OWN_RANGES[r];
                for layers 0/1 follow with the next layer's projection of
                the same rows, for the last layer write y output rows."""
                last = li == 2
                (o_lo, o_hi) = OWN_RANGES[r]
                dst_hT = hT2 if li % 2 == 0 else hT
                for b0 in range(o_lo, o_hi, 8):
                    nb = min(8, o_hi - b0)
                    osum = prange_pool.tile([128, 8, 132], f32, tag="osum")
                    nc.sync.dma_start(
                        out=osum[:, 0:nb, :],
                        in_=own_sum[b0 * 128:(b0 + nb) * 128, :]
                            .rearrange("(t p) c -> p t c", p=128))
                    den = prange_pool.tile([128, 8, 4], f32, tag="den")
                    nc.vector.tensor_scalar_max(den[:, 0:nb, :],
                                                osum[:, 0:nb, 128:132], EPS)
                    rec = prange_pool.tile([128, 8, 4, 1], f32, tag="rec")
                    nc.vector.reciprocal(rec[:, 0:nb, :, 0], den[:, 0:nb, :])
                    if not last:
                        o2 = prange_pool.tile([128, 8, 128], f32, tag="o2")
                        nc.vector.tensor_tensor(
                            out=o2[:, 0:nb, :]
                                .rearrange("p t (h d) -> p t h d", h=4),
                            in0=osum[:, 0:nb, 0:128]
                                .rearrange("p t (h d) -> p t h d", h=4),
                            in1=rec[:, 0:nb, :, :]
                                .to_broadcast([128, nb, 4, 32]),
                            op=OP.mult)
                        nc.vector.tensor_tensor(
                            out=o2[:, 0:nb, :], in0=o2[:, 0:nb, :],
                            in1=brep_sb[li][:]
                                .rearrange("p (t c) -> p t c", t=1)
                                .to_broadcast([128, nb, 128]),
                            op=OP.add)
                        # ELU via scalar engine: exn = exp(-relu(-x)) =
                        # exp(min(x,0)); o2 = relu(x) + exn - 1
                        exn = prange_pool.tile([128, 8, 128], f32, tag="exn")
                        nc.scalar.activation(exn[:, 0:nb, :], o2[:, 0:nb, :],
                                             AF.Relu, scale=-1.0)
                        nc.scalar.activation(exn[:, 0:nb, :], exn[:, 0:nb, :],
                                             AF.Exp, scale=-1.0)
                        nc.scalar.activation(o2[:, 0:nb, :], o2[:, 0:nb, :],
                                             AF.Relu)
                        nc.vector.tensor_tensor(out=o2[:, 0:nb, :],
                                                in0=o2[:, 0:nb, :],
                                                in1=exn[:, 0:nb, :],
                                                op=OP.add)
                        nc.scalar.activation(o2[:, 0:nb, :], o2[:, 0:nb, :],
                                             AF.Copy, bias=-1.0)
                        for t in range(nb):
                            tp = prj_ps.tile([128, 136], f32, space="PSUM",
                                             tag="aux")
                            nc.tensor.matmul(tp[:, 0:128], lhsT=o2[:, t, :],
                                             rhs=ident_sb[:], start=True,
                                             stop=True)
                            nc.scalar.activation(
                                dst_hT[:, (b0 + t) * 128:(b0 + t + 1) * 128],
                                tp[:, 0:128], AF.Copy)
                    else:
                        r4 = prange_pool.tile([128, 8, 4, 32], f32, tag="r4")
                        nc.vector.tensor_tensor(
                            out=r4[:, 0:nb, :, :],
                            in0=osum[:, 0:nb, 0:128]
                                .rearrange("p t (h d) -> p t h d", h=4),
                            in1=rec[:, 0:nb, :, :]
                                .to_broadcast([128, nb, 4, 32]),
                            op=OP.mult)
                        r1 = prange_pool.tile([128, 8, 32], f32, tag="r1")
                        nc.vector.tensor_tensor(out=r1[:, 0:nb, :],
                                                in0=r4[:, 0:nb, 0, :],
                                                in1=r4[:, 0:nb, 1, :],
                                                op=OP.add)
                        r2 = prange_pool.tile([128, 8, 32], f32, tag="r2")
                        nc.vector.tensor_tensor(out=r2[:, 0:nb, :],
                                                in0=r4[:, 0:nb, 2, :],
                                                in1=r4[:, 0:nb, 3, :],
                                                op=OP.add)
                        nc.vector.tensor_tensor(out=r1[:, 0:nb, :],
                                                in0=r1[:, 0:nb, :],
                                                in1=r2[:, 0:nb, :], op=OP.add)
                        nc.vector.tensor_scalar_mul(r1[:, 0:nb, :],
                                                    r1[:, 0:nb, :], 0.25)
                        nc.vector.tensor_tensor(
                            out=r1[:, 0:nb, :], in0=r1[:, 0:nb, :],
                            in1=brep_sb[li][:]
                                .rearrange("p (t c) -> p t c", t=1)
                                .to_broadcast([128, nb, 32]),
                            op=OP.add)
                        nfull = nb if (b0 + nb) * 128 <= NPC else nb - 1
                        if nfull > 0:
                            nc.sync.dma_start(
                                out=y_out[b0 * 128:(b0 + nfull) * 128, :]
                                    .rearrange("(t p) c -> p t c", p=128),
                                in_=r1[:, 0:nfull, :])
                        if nfull < nb:
                            rem = NPC - (b0 + nfull) * 128
                            nc.sync.dma_start(
                                out=y_out[(b0 + nfull) * 128:NPC, :],
                                in_=r1[0:rem, nfull, :])
                if not last:
                    proj_range(li + 1, dst_hT, r, prange_pool, prj_ps)
                    # table AG pieces: r=0,1 fire mid-pass-B (CC has slack
                    # there); r=2,3 are deferred past the RS pieces so they
                    # never delay the boundary-critical ReduceScatters.
                    if r <= 1:
                        table_ag_piece(li + 1, r)
                    elif r == 3:
                        table_ag_piece(li + 1, 2)
                        table_ag_piece(li + 1, 3)
                        # AG pieces 0/1 fired mid-pass-B and are done;
                        # restripe them now. Pieces 2/3 restripe during the
                        # next layer's pass A (front_hook).
                        restripe_piece(li + 1, 0)
                        restripe_piece(li + 1, 1)

            # ---- layer 0 init: load xT + streamed projection ----
            prefetch_masks("A0", schedA, smTA_in)
            with tc.tile_pool(name="prj0", bufs=2) as p0p, \
                 tc.tile_pool(name="prj0ps", bufs=3, space="PSUM") as p0ps:
                for r, (o_lo, o_hi) in enumerate(OWN_RANGES):
                    nc.sync.dma_start(
                        out=hT[:, o_lo * 128:o_hi * 128],
                        in_=xT_in[:, o_lo * 128:o_hi * 128])
                    proj_range(0, hT, r, p0p, p0ps)
                    table_ag_piece(0, r)
                for r in range(len(OWN_RANGES)):
                    restripe_piece(0, r)

            for li in range(3):
                last = li == 2

                with tc.tile_pool(name=f"gt{li}", bufs=4) as gp, \
                     tc.tile_pool(name=f"ms{li}", bufs=4) as mp, \
                     tc.tile_pool(name=f"ex{li}", bufs=4) as xp, \
                     tc.tile_pool(name=f"pb{li}", bufs=4) as pbp, \
                     tc.tile_pool(name=f"pa{li}", bufs=4) as pap, \
                     tc.tile_pool(name=f"sg{li}", bufs=2,
                                  space="PSUM") as sgps, \
                     tc.tile_pool(name=f"aux{li}", bufs=3,
                                  space="PSUM") as auxps, \
                     tc.tile_pool(name=f"er{li}", bufs=3,
                                  space="PSUM") as erps:

                    def edge_pass(sched, idx_sb, smT_in, tab, passB,
                                  mkey, on_back_done=None, front_hook=None):
                        seg_tiles = {}
                        pa_tiles = {}
                        rs_left = [len(bl) for (bl, *_r) in RS_PLAN]
                        rs_fired_at = [None] * len(RS_PLAN)
                        state = {}

                        def emit_front(ci):
                            # gather + mask stream + er matmuls for chunk ci;
                            # then derive sm = transpose(smT) on the PE for
                            # the accumulate matmuls two chunks later.
                            (t0, nt, m0, nm) = sched["chunks"][ci]
                            g = gp.tile([128, CHUNK, TCOLS], bf16, tag="g")
                            nc.gpsimd.dma_gather(
                                out_ap=g[:, 0:nt, :], in_ap=tab[:, :],
                                idxs_ap=idx_sb[:, t0 * 8:(t0 + nt) * 8],
                                num_idxs=nt * 128, num_idxs_reg=nt * 128,
                                elem_size=TCOLS, single_packet=False)
                            smT = load_masks(mkey, sched, smT_in, ci)
                            er_ps = erps.tile([128, CHUNK, 4], f32,
                                              space="PSUM", tag="er_ps")
                            for t in range(nt):
                                for (ms, b, ef, el_, _sf, _sl) in \
                                        sched["tiles"][t0 + t]:
                                    lm = ms - m0
                                    nc.tensor.matmul(
                                        er_ps[:, t, :],
                                        lhsT=smT[:, lm * 128:(lm + 1) * 128],
                                        rhs=er_sb[li % 2][int(RNG_OF[b])][
                                            :, int(LROW_OF[b]), :],
                                        start=ef, stop=el_)
                            sm = mkp.tile([128, NM_MAX * 128], bf16, tag="sm")
                            for lm in range(nm):
                                st = auxps.tile([128, 136], f32,
                                                space="PSUM", tag="aux")
                                nc.tensor.matmul(
                                    st[:, 0:128],
                                    lhsT=smT[:, lm * 128:(lm + 1) * 128],
                                    rhs=identb_sb[:], start=True, stop=True)
                                nc.scalar.activation(
                                    sm[:, lm * 128:(lm + 1) * 128],
                                    st[:, 0:128], AF.Copy)
                            state[ci] = (t0, nt, m0, g, sm, er_ps)

                        def emit_back(ci):
                            (t0, nt, m0, g, sm, er_ps) = state.pop(ci)
                            for g0 in range(0, nt, GROUP):
                                gl = min(GROUP, nt - g0)
                                e4 = xp.tile([128, GROUP, 4], f32, tag="e4")
                                nc.vector.tensor_tensor(
                                    out=e4[:, 0:gl, :],
                                    in0=g[:, g0:g0 + gl, 128:132],
                                    in1=er_ps[:, g0:g0 + gl, :], op=OP.add)
                                lr = xp.tile([128, GROUP, 4], f32, tag="lr")
                                nc.scalar.activation(lr[:, 0:gl, :],
                                                     e4[:, 0:gl, :],
                                                     AF.Prelu, alpha=NEG)
                                ex4 = xp.tile([128, GROUP, 4, 1], f32,
                                              tag="ex4")
                                nc.scalar.activation(ex4[:, 0:gl, :, 0],
                                                     lr[:, 0:gl, :], AF.Exp)
                                m4 = mp.tile([128, GROUP, 132], bf16, tag="m4")
                                nc.scalar.activation(m4[:, 0:gl, 128:132],
                                                     ex4[:, 0:gl, :, 0],
                                                     AF.Copy)
                                nc.vector.tensor_tensor(
                                    out=m4[:, 0:gl, 0:128],
                                    in0=g[:, g0:g0 + gl, 0:128],
                                    in1=ex4[:, 0:gl, :, :]
                                        .to_broadcast([128, gl, 4, 32]),
                                    op=OP.mult)
                                for t in range(gl):
                                    for (ms, b, _ef, _el, sf, sl) in \
                                            sched["tiles"][t0 + g0 + t]:
                                        lm = ms - m0
                                        if sf:
                                            seg_tiles[b] = sgps.tile(
                                                [128, 132], f32, space="PSUM",
                                                tag="seg",
                                                name=f"seg{li}_{passB}_{b}")
                                            if passB:
                                                pa = pap.tile([128, 132], f32,
                                                              tag="pa",
                                                              name=f"pa{li}_{b}")
                                                ri_ = int(RNG_OF[b])
                                                lr = int(LROW_OF[b])
                                                nc.sync.dma_start(
                                                    out=pa[:],
                                                    in_=partialA[ri_][
                                                        lr * 128:
                                                        (lr + 1) * 128, :])
                                                pa_tiles[b] = pa
                                        nc.tensor.matmul(
                                            seg_tiles[b][:],
                                            lhsT=sm[:, lm * 128:(lm + 1) * 128],
                                            rhs=m4[:, t, :],
                                            start=sf, stop=sl)
                                        if sl:
                                            pb = pbp.tile([128, 132], f32,
                                                          tag="pb")
                                            ri_ = int(RNG_OF[b])
                                            lr = int(LROW_OF[b])
                                            if passB:
                                                nc.vector.tensor_tensor(
                                                    out=pb[:],
                                                    in0=seg_tiles[b][:],
                                                    in1=pa_tiles.pop(b)[:],
                                                    op=OP.add)
                                                nc.sync.dma_start(
                                                    out=partial[ri_][
                                                        lr * 128:
                                                        (lr + 1) * 128, :],
                                                    in_=pb[:])
                                                (bl, p_lo, p_hi, so_lo,
                                                 so_hi) = RS_PLAN[ri_]
                                                rs_left[ri_] -= 1
                                                if rs_left[ri_] == 0:
                                                    if not no_cc:
                                                        nc.gpsimd.\
                                                            collective_compute(
                                                            "ReduceScatter",
                                                            mybir.AluOpType.add,
                                                            replica_groups=
                                                            groups_pair,
                                                            ins=[partial[ri_]
                                                                 [:, :]],
                                                            outs=[own_sum[
                                                                so_lo:so_hi,
                                                                :]])
                                                    else:
                                                        sz = so_hi - so_lo
                                                        nc.sync.dma_start(
                                                            out=own_sum[
                                                                so_lo:so_hi,
                                                                :],
                                                            in_=partial[ri_][
                                                                0:sz, :])
                                                    rs_fired_at[ri_] = ci
                                            else:
                                                nc.scalar.activation(
                                                    pb[:], seg_tiles[b][:],
                                                    AF.Copy)
                                                nc.sync.dma_start(
                                                    out=partialA[ri_][
                                                        lr * 128:
                                                        (lr + 1) * 128, :],
                                                    in_=pb[:])
                                            seg_tiles.pop(b)
                            if on_back_done is not None:
                                on_back_done(ci, rs_fired_at)

                        nchunk = len(sched["chunks"])
                        AHEAD = 2
                        for ci in range(min(AHEAD, nchunk)):
                            emit_front(ci)
                            if front_hook is not None:
                                front_hook(ci)
                        for ci in range(AHEAD, nchunk):
                            emit_front(ci)
                            if front_hook is not None:
                                front_hook(ci)
                            emit_back(ci - AHEAD)
                        for ci in range(max(0, nchunk - AHEAD), nchunk):
                            emit_back(ci)

                    # post/proj streaming behind the split RS pieces.
                    # RS_DELAY chunks of slack before emitting each range's
                    # post ops so engine queues never stall on the RS sem.
                    RS_DELAY = [3, 2, 2, 0]
                    posted = [False] * len(RS_PLAN)
                    prefA_done = [False]

                    with tc.tile_pool(name=f"pr{li}", bufs=2) as prp:
                        prps = auxps

                        def on_back_done(ci, rs_fired_at):
                            nchunk = len(schedB["chunks"])
                            # prefetch next layer's pass-A masks mid-pass-B
                            if not last and not prefA_done[0] and \
                                    ci >= nchunk // 2:
                                prefetch_masks(f"A{li + 1}", schedA, smTA_in)
                                prefA_done[0] = True
                            for ri in range(len(RS_PLAN)):
                                if posted[ri] or rs_fired_at[ri] is None:
                                    continue
                                ready = (rs_fired_at[ri] + RS_DELAY[ri] <= ci
                                         or ci == nchunk - 1)
                                if ready and (ri == 0 or posted[ri - 1]):
                                    post_range(li, ri, prp, prps)
                                    posted[ri] = True

                        def front_hook_A(ci):
                            if li > 0 and ci == 5:
                                restripe_piece(li, 2)
                            elif li > 0 and ci == 7:
                                restripe_piece(li, 3)

                        edge_pass(schedA, idxA_sb, smTA_in, ag_own,
                                  False, f"A{li}", front_hook=front_hook_A)
                        prefetch_masks(f"B{li}", schedB, smTB_in)
                        edge_pass(schedB, idxB_sb, smTB_in,
                                  table[li % 2], True, f"B{li}", on_back_done)
                        for ri in range(len(RS_PLAN)):
                            if not posted[ri]:
                                post_range(li, ri, prp, prps)
                                posted[ri] = True
    nc.finalize()
    return nc


def _make_consts(W0, al0, ar0, b0, W1, al1, ar1, b1, W2, al2, ar2, b2):
    consts = {}
    for li, (W, al, ar, b) in enumerate(
            [(W0, al0, ar0, b0), (W1, al1, ar1, b1), (W2, al2, ar2, b2)]):
        consts[f"Waug{li}"] = _augment(np.asarray(W, np.float32),
                                       np.asarray(al, np.float32),
                                       np.asarray(ar, np.float32)).astype(
                                           ml_dtypes.bfloat16)
        b = np.asarray(b, np.float32)
        if li < 2:
            consts[f"brep{li}"] = np.tile(b.reshape(1, 128), (128, 1))
        else:
            consts[f"brep{li}"] = np.tile(b.reshape(H, D).mean(0).reshape(1, D),
                                          (128, 1))
    dummy = np.zeros((1, TCOLS), ml_dtypes.bfloat16)
    dummy[0, 128:132] = ml_dtypes.bfloat16(-1e30)
    consts["dummyrow"] = dummy
    return consts


def _in_maps(x):
    cores = _cache["pre"][0]
    x = np.asarray(x, dtype=np.float32)
    in_maps = []
    for c in range(NCORE):
        lo, hi = _own_rows(c)
        xT = np.zeros((128, OWN), ml_dtypes.bfloat16)
        xT[:, 0:NPC] = x[lo:hi].T.astype(ml_dtypes.bfloat16)
        cc = cores[c]
        in_maps.append(dict(xT=xT, idxA=cc["idxA"], idxB=cc["idxB"],
                            smTA=np.asarray(cc["smTA"]),
                            smTB=np.asarray(cc["smTB"])))
    return in_maps


def kernel(x, src, dst, W0, al0, ar0, b0, W1, al1, ar1, b1, W2, al2, ar2, b2):
    from concourse.bass_utils import run_bass_kernel_spmd

    key = (hash(np.asarray(src).tobytes()) ^ hash(np.asarray(dst).tobytes()))
    if "pre" not in _cache or _cache.get("prekey") != key:
        _cache["pre"] = _preprocess(src, dst)
        _cache["prekey"] = key
    cores, schedA, schedB = _cache["pre"]

    consts = _make_consts(W0, al0, ar0, b0, W1, al1, ar1, b1, W2, al2, ar2, b2)

    ck = key ^ hash(consts["Waug0"].tobytes())
    if "nc" not in _cache or _cache.get("nckey") != ck:
        _cache["nc"] = _build(schedA, schedB, consts)
        _cache["nckey"] = ck
    nc = _cache["nc"]

    in_maps = _in_maps(x)
    r = run_bass_kernel_spmd(nc, in_maps, list(range(NCORE)))
    y = np.zeros((N, D), np.float32)
    for c in range(NCORE):
        lo, hi = _own_rows(c)
        y[lo:hi] = r.results[c]["y"]
    return y


# revision 66
# speedup vs baseline: 1.0896x; 1.0896x over previous
"""3-layer GAT on 8 Trainium2 NeuronCores (Bass/Tile).

Sharding: 2D graph partition. Pair q = cores {2q, 2q+1} aggregates the dst
nodes of strips [q*6250,(q+1)*6250) and [25000+q*6250, 25000+(q+1)*6250);
even cores take edges with src < 25000, odd cores the rest. Node ownership:
core 2k owns rows [k*6250,(k+1)*6250), core 2k+1 owns [25000+k*6250, ...).

Per layer: each core projects its own rows (feat|el|er via an augmented
weight matrix) into a local gather table, then runs the edge phase in two
passes: pass A covers edges whose source is one of the core's own rows and
gathers from the local table while the quad AllGather of the full src-half
table is still in flight; pass B covers the remaining edges and gathers
from the AllGathered table. Edges are packed into 128-wide tiles grouped
by pairs of 128-dst blocks (a tile may straddle the two blocks; the
host-precomputed one-hot masks select membership). Per-edge er comes from
a transposed one-hot matmul against SBUF-resident per-block er rows (no
second gather). Messages are accumulated per dst block by one-hot-mask
matmuls into PSUM; pass B adds pass A's partial sums back in.

The dst pair-groups are processed in 4 segments; the pairwise
ReduceScatter of partial sums is split into 4 matching pieces (the
partial-row permutation interleaves [A-range | B-range] per piece so each
piece is contiguous), each fired as soon as its blocks complete mid-pass-B.
Post-processing (divide/bias/ELU, head-mean on the last layer) and the
next layer's projection are streamed per own-block range behind each RS
piece, so almost the whole layer boundary hides under pass B. The er
AllGather is split per range the same way (er_tab shares the partial-row
permutation, keeping each piece contiguous); mask loads for the first
chunks of each pass are prefetched during the previous pass. Gather calls
carry trailing -1 indices so the Q7 descriptor generator truncates padded
tail slots.
"""

import numpy as np
import ml_dtypes

N = 50000
E = 800000
F = 128                  # input feats and hidden width (4 heads x 32)
H = 4
D = 32
NEG = 0.2
NCORE = 8
NPC = 6250               # nodes owned per core
OWN = 6272               # 49*128, padded own rows
OWNBLK = 49
PAIR = 12544             # 98*128 dst slots per pair
NBLK = 98
NGRP = 49                # pair-groups of 2 blocks
HALF = 25088             # 4*OWN rows per src-half table
TROWS = 25216            # HALF + 128 (dummy row at HALF)
DUMMY = HALF
TCOLS = 256              # bf16 cols: feat(128) | el(4) | pad
PCOLS = 132              # packed AllGather row: feat(128) | el(4)
CHUNK = 20               # max tiles per dma_gather call
GROUP = 8                # tiles per vector-op batch
EPS = 1e-30

# Own-block ranges: post/proj are streamed per range, each behind its own
# ReduceScatter piece. The last range is small so the layer-boundary chain
# (last RS piece -> post -> proj -> table write) is short.
OWN_RANGES = [(0, 22), (22, 34), (34, 45), (45, 49)]

# pass-B pair-group order: 4 segments, segment r completes the blocks of
# OWN_RANGES[r] (A side: blocks lo..hi-1, B side: 49+lo..49+hi-1), so each
# ReduceScatter piece fires as early as possible. Straddle groups whose
# later-range block completes early are harmless (counters are per block).
GORDER_B = (list(range(0, 11)) + list(range(24, 36)) +    # seg 1
            list(range(11, 17)) + list(range(36, 42)) +   # seg 2
            list(range(17, 23)) + list(range(42, 47)) +   # seg 3
            [23, 47, 48])                                 # seg 4

# pass-A pair-group order: sorted by the er-AllGather piece each group
# needs (max over its two blocks), so groups needing the late pieces sit at
# the end of pass A and never stall the in-order engine queues.
GORDER_A = (list(range(0, 11)) + list(range(25, 35)) +    # er range 1
            list(range(11, 17)) + list(range(35, 41)) +   # er range 2
            list(range(17, 22)) + list(range(41, 47)) +   # er range 3
            [22, 23, 24, 47, 48])                         # er range 4

# block -> partial-row-block permutation: [A1 B1 | A2 B2 | A3 B3 | A4 B4]
# where Ar/Br are the A/B-side blocks of OWN_RANGES[r]. The same layout is
# used for er_tab so the split pair-AllGather outputs stay contiguous.
ROW_OF = np.empty(NBLK, np.int64)
RNG_OF = np.empty(NBLK, np.int64)    # block -> range index
LROW_OF = np.empty(NBLK, np.int64)   # block -> row-block within its range
RS_PLAN = []             # (blockset, p_lo, p_hi, o_lo, o_hi) per range
_base = 0
for _ri, (_lo, _hi) in enumerate(OWN_RANGES):
    _sz = _hi - _lo
    ROW_OF[_lo:_hi] = _base + np.arange(_sz)
    ROW_OF[49 + _lo:49 + _hi] = _base + _sz + np.arange(_sz)
    RNG_OF[_lo:_hi] = _ri
    RNG_OF[49 + _lo:49 + _hi] = _ri
    LROW_OF[_lo:_hi] = np.arange(_sz)
    LROW_OF[49 + _lo:49 + _hi] = _sz + np.arange(_sz)
    _blocks = frozenset(range(_lo, _hi)) | frozenset(range(49 + _lo, 49 + _hi))
    RS_PLAN.append((_blocks, _base * 128, (_base + 2 * _sz) * 128,
                    _lo * 128, _hi * 128))
    _base += 2 * _sz
assert _base == NBLK

# Range-major layout of the quad-AllGathered src-half table: range r holds
# its 4 ranks' own-row stripes contiguously, so the table AllGather (and
# the local 264B->512B re-stripe) splits into 4 contiguous pieces.
T_PBASE = []
_tb = 0
for _lo, _hi in OWN_RANGES:
    T_PBASE.append(_tb)
    _tb += 4 * (_hi - _lo) * 128
assert _tb == HALF
# own-local row -> range-major table row offset (within one rank's stripe
# the rows keep own-local order; rank k of range r starts at
# T_PBASE[r] + k * sz_r * 128).
_OROW_RANGE = np.empty(OWN, np.int64)   # own row -> range idx
_OROW_LOCAL = np.empty(OWN, np.int64)   # own row -> row within range stripe
for _ri, (_lo, _hi) in enumerate(OWN_RANGES):
    _OROW_RANGE[_lo * 128:_hi * 128] = _ri
    _OROW_LOCAL[_lo * 128:_hi * 128] = np.arange((_hi - _lo) * 128)

_cache = {}


def _schedule(cnt, gorder):
    """Core-uniform tile/mask schedule for one pass.

    cnt: [NCORE, NBLK] per-core per-block edge counts.
    Returns dict with T, nmask, chunks, tiles (per tile: list of
    (mslot, block, er_first, er_last, sc_first, sc_last)).
    """
    GORDER = gorder
    n0 = cnt[:, 0::2]                      # [NCORE, NGRP]
    n1 = cnt[:, 1::2]
    TP = np.maximum(1, np.ceil((n0 + n1).max(axis=0) / 128).astype(np.int64))

    base_tile = {}
    acc = 0
    for g in GORDER:
        base_tile[g] = acc
        acc += int(TP[g])
    T = acc

    tiles = []            # per tile: list of [mslot, block]
    tile_group = []
    # which (tile-in-group, block-parity) pairs are needed on any core; ensure
    # every block gets at least one occurrence (tile 0 fallback)
    need = {}
    for g in GORDER:
        for i in range(int(TP[g])):
            need[(g, i, 0)] = bool((n0[:, g] > 128 * i).any())
            need[(g, i, 1)] = bool(
                ((n0[:, g] < 128 * (i + 1)) &
                 (n0[:, g] + n1[:, g] > 128 * i)).any())
        if not any(need[(g, i, 0)] for i in range(int(TP[g]))):
            need[(g, 0, 0)] = True
        if not any(need[(g, i, 1)] for i in range(int(TP[g]))):
            need[(g, 0, 1)] = True
    mslot = 0
    for g in GORDER:
        for i in range(int(TP[g])):
            ml = []
            if need[(g, i, 0)]:
                ml.append([mslot, 2 * g])
                mslot += 1
            if need[(g, i, 1)]:
                ml.append([mslot, 2 * g + 1])
                mslot += 1
            assert ml
            tiles.append(ml)
            tile_group.append(g)
    nmask = mslot

    # per-block first/last occurrence
    occ = {}
    for ti, ml in enumerate(tiles):
        for m in ml:
            occ.setdefault(m[1], []).append((ti, m[0]))
    first = {b: o[0] for b, o in occ.items()}
    last = {b: o[-1] for b, o in occ.items()}
    sched_tiles = []
    for ti, ml in enumerate(tiles):
        entry = []
        for k, (ms, b) in enumerate(ml):
            entry.append((ms, b,
                          k == 0, k == len(ml) - 1,
                          first[b] == (ti, ms), last[b] == (ti, ms)))
        sched_tiles.append(entry)

    # chunks aligned to pair-group boundaries, up to CHUNK tiles
    chunks = []
    t0 = 0
    ti = 0
    for g in GORDER:
        ti += int(TP[g])
        nxt = None
        gi = GORDER.index(g)
        if gi + 1 < len(GORDER):
            nxt = int(TP[GORDER[gi + 1]])
        if nxt is None or ti - t0 + nxt > CHUNK:
            m0 = min(m[0] for m in sched_tiles[t0]) if sched_tiles[t0] else 0
            mend = max(m[0] for m in sched_tiles[ti - 1]) + 1
            chunks.append((t0, ti - t0, m0, mend - m0))
            t0 = ti
    assert t0 == T
    return dict(T=T, nmask=nmask, chunks=chunks, tiles=sched_tiles,
                tile_group=tile_group, base_tile=base_tile, TP=TP,
                gorder=gorder)


def _wrap16(a):
    # value i of each 128-group at [i%16, i//16], replicated per 16 rows
    t = a.reshape(-1, 128)                     # [T, 128]
    w = t.reshape(t.shape[0], 8, 16)           # [T, 8, 16]
    w = w.transpose(2, 0, 1).reshape(16, -1)   # [16, T*8]
    return np.tile(w, (8, 1)).astype(np.int16)  # [128, T*8]


def _core_pass_arrays(sched, rloc_e, rows_e, pad_row, trunc_from_chunk):
    """Build idx + mask streams for one (core, pass).

    rloc_e: pair-local dst row per edge; rows_e: gather-table row per edge.
    Chunks with index >= trunc_from_chunk get their trailing padded idx
    slots set to -1 (the Q7 truncates them); earlier chunks keep the dummy
    row so first-touch SBUF tiles never expose uninitialized data.
    """
    T, nmask = sched["T"], sched["nmask"]
    base_tile = sched["base_tile"]
    GORDER = sched["gorder"]
    # group rank of each edge
    grank_of = np.empty(NGRP, np.int64)
    for r, g in enumerate(GORDER):
        grank_of[g] = r
    pg = rloc_e // 256
    gr = grank_of[pg]
    order = np.lexsort((rloc_e, gr))
    rloc_s = rloc_e[order]
    rows_s = rows_e[order]
    gr_s = gr[order]
    # position within group
    starts = np.searchsorted(gr_s, np.arange(len(GORDER)))
    pos_in_group = np.arange(len(gr_s)) - starts[gr_s]
    base128 = np.array([base_tile[GORDER[r]] * 128
                        for r in range(len(GORDER))], np.int64)
    s_glob = base128[gr_s] + pos_in_group

    idx = np.full(T * 128, pad_row, np.int64)
    idx[s_glob] = rows_s
    real = np.zeros(T * 128, bool)
    real[s_glob] = True
    for ci, (t0, nt, _m0, _nm) in enumerate(sched["chunks"]):
        if ci < trunc_from_chunk:
            continue
        a, b = t0 * 128, (t0 + nt) * 128
        nz = np.flatnonzero(real[a:b])
        last = nz[-1] if len(nz) else -1
        idx[a + last + 1:b] = -1

    # mask slot lookup per (tile, block-parity)
    mslot_of = np.full((T, 2), -1, np.int64)
    for ti, ml in enumerate(sched["tiles"]):
        g = sched["tile_group"][ti]
        for (ms, b, *_fl) in ml:
            mslot_of[ti, b - 2 * g] = ms
    ti_e = s_glob // 128
    e_e = s_glob % 128
    b_e = rloc_s // 128
    s128_e = rloc_s % 128
    par = b_e - 2 * np.array(sched["tile_group"])[ti_e]
    ms_e = mslot_of[ti_e, par]
    assert (ms_e >= 0).all()

    smatTw = np.zeros((128, nmask * 128), ml_dtypes.bfloat16)
    smatTw[s128_e, ms_e * 128 + e_e] = 1
    return _wrap16(idx), smatTw


def _preprocess(src, dst):
    src = np.asarray(src).astype(np.int64)
    dst = np.asarray(dst).astype(np.int64)
    q = np.where(dst < 25000, dst // NPC, (dst - 25000) // NPC)
    s = (src >= 25000).astype(np.int64)
    core_of = 2 * q + s
    rloc = np.where(dst < 25000, dst - q * NPC, OWN + (dst - 25000 - q * NPC))
    ks = np.where(src < 25000, src // NPC, (src - 25000) // NPC)
    olocal = np.where(src < 25000, src - ks * NPC, (src - 25000) - ks * NPC)
    _ri = _OROW_RANGE[olocal]
    _szr = np.array([(hi - lo) * 128 for lo, hi in OWN_RANGES], np.int64)
    tloc = (np.array(T_PBASE, np.int64)[_ri] + ks * _szr[_ri]
            + _OROW_LOCAL[olocal])
    own = ks == q
    ownrow = np.where(src < 25000, src - ks * NPC, src - 25000 - ks * NPC)
    blk = rloc // 128

    cntA = np.zeros((NCORE, NBLK), np.int64)
    cntB = np.zeros((NCORE, NBLK), np.int64)
    for c in range(NCORE):
        m = core_of == c
        cntA[c] = np.bincount(blk[m & own], minlength=NBLK)
        cntB[c] = np.bincount(blk[m & ~own], minlength=NBLK)
    schedA = _schedule(cntA, GORDER_A)
    schedB = _schedule(cntB, GORDER_B)

    cores = []
    for c in range(NCORE):
        m = core_of == c
        mA = m & own
        mB = m & ~own
        idxA, smTA = _core_pass_arrays(schedA, rloc[mA], ownrow[mA],
                                       OWN, 10**9)
        idxB, smTB = _core_pass_arrays(schedB, rloc[mB], tloc[mB],
                                       DUMMY, 10**9)
        cores.append(dict(idxA=idxA, smTA=smTA, idxB=idxB, smTB=smTB))
    return cores, schedA, schedB


def _own_rows(c):
    k = c // 2
    if c % 2 == 0:
        return k * NPC, (k + 1) * NPC
    return 25000 + k * NPC, 25000 + (k + 1) * NPC


def _augment(W, al, ar):
    dout = W.shape[1] // H
    Wal = np.stack([W[:, h * dout:(h + 1) * dout] @ al[h] for h in range(H)], 1)
    War = np.stack([W[:, h * dout:(h + 1) * dout] @ ar[h] for h in range(H)], 1)
    return np.concatenate([W, Wal, War], 1).astype(np.float32)  # [128, 136]


def _build(schedA, schedB, consts, no_cc=False):
    import concourse.bass as bass
    import concourse.bacc as bacc
    import concourse.tile as tile
    from concourse import mybir
    from concourse.library_config import mlp

    f32 = mybir.dt.float32
    bf16 = mybir.dt.bfloat16
    i16 = mybir.dt.int16
    AF = mybir.ActivationFunctionType
    OP = mybir.AluOpType

    TA, TB = schedA["T"], schedB["T"]
    NMA, NMB = schedA["nmask"], schedB["nmask"]
    NM_MAX = max(max(nm for (_, _, _, nm) in schedA["chunks"]),
                 max(nm for (_, _, _, nm) in schedB["chunks"]))

    nc = bacc.Bacc(num_devices=NCORE)
    xT_in = nc.declare_dram_parameter("xT", [128, OWN], bf16, isOutput=False)
    idxA_in = nc.declare_dram_parameter("idxA", [128, TA * 8], i16,
                                        isOutput=False)
    idxB_in = nc.declare_dram_parameter("idxB", [128, TB * 8], i16,
                                        isOutput=False)
    smTA_in = nc.declare_dram_parameter("smTA", [128, NMA * 128], bf16,
                                        isOutput=False)
    smTB_in = nc.declare_dram_parameter("smTB", [128, NMB * 128], bf16,
                                        isOutput=False)
    y_out = nc.declare_dram_parameter("y", [NPC, D], f32, isOutput=True)

    with tile.TileContext(nc) as tc:
        with tc.tile_pool(name="persist", bufs=1) as pp, \
             tc.tile_pool(name="mk", bufs=3) as mkp, \
             tc.tile_pool(name="dram", bufs=1, space="DRAM") as dp:
            nc.gpsimd.load_library(mlp)

            # ---- persistent SBUF state ----
            idxA_sb = pp.tile([128, TA * 8], i16)
            nc.sync.dma_start(out=idxA_sb[:], in_=idxA_in[:, :])
            idxB_sb = pp.tile([128, TB * 8], i16)
            nc.sync.dma_start(out=idxB_sb[:], in_=idxB_in[:, :])
            hT = pp.tile([128, OWN], bf16)
            hT2 = pp.tile([128, OWN], bf16)

            ident_h = nc.inline_tensor(np.eye(128, dtype=np.float32),
                                       name="ident")
            ident_sb = pp.tile([128, 128], f32)
            nc.sync.dma_start(out=ident_sb[:], in_=ident_h[:, :])
            identb_h = nc.inline_tensor(np.eye(128).astype(ml_dtypes.bfloat16),
                                        name="identb")
            identb_sb = pp.tile([128, 128], bf16)
            nc.sync.dma_start(out=identb_sb[:], in_=identb_h[:, :])

            waug_sb = []
            brep_sb = []
            for li in range(3):
                wh = nc.inline_tensor(consts[f"Waug{li}"], name=f"waug{li}")
                wt = pp.tile([128, 136], bf16, name=f"waug_sb{li}")
                nc.sync.dma_start(out=wt[:], in_=wh[:, :])
                waug_sb.append(wt)
                bh = nc.inline_tensor(consts[f"brep{li}"], name=f"brep{li}")
                bt = pp.tile([128, consts[f"brep{li}"].shape[1]], f32,
                             name=f"brep_sb{li}")
                nc.sync.dma_start(out=bt[:], in_=bh[:, :])
                brep_sb.append(bt)

            dummy_h = nc.inline_tensor(consts["dummyrow"], name="dummyrow")

            # ---- DRAM scratch ----
            # partialA/partial are split per own-block range so pass-B loads
            # of a range only depend on that range's pass-A writes (and the
            # RS pieces only on their own range's writes).
            # The gather table is double buffered by layer parity (the next
            # layer's split AllGather pieces land while the current layer's
            # pass B still gathers from the old table). The AllGather moves
            # packed 264B rows; a local re-stripe expands to the 512B-stride
            # layout the gather needs.
            table = [dp.tile([TROWS, TCOLS], bf16, name=f"table{p}")
                     for p in range(2)]
            table_pack = dp.tile([HALF, PCOLS], bf16)
            er_tab = dp.tile([PAIR, 4], bf16)
            ag_own = dp.tile([OWN + 128, TCOLS], bf16)
            ag_pack = dp.tile([OWN, PCOLS], bf16)
            ag_er = dp.tile([OWN, 4], bf16)
            partialA = [dp.tile([(p_hi - p_lo), 132], f32,
                                name=f"partialA{ri}")
                        for ri, (_b, p_lo, p_hi, _o, _o2) in
                        enumerate(RS_PLAN)]
            partial = [dp.tile([(p_hi - p_lo), 132], f32, name=f"partial{ri}")
                       for ri, (_b, p_lo, p_hi, _o, _o2) in
                       enumerate(RS_PLAN)]
            own_sum = dp.tile([OWN, 132], f32)

            nc.sync.dma_start(out=table[0][DUMMY:DUMMY + 1, :],
                              in_=dummy_h[:, :])
            nc.sync.dma_start(out=table[1][DUMMY:DUMMY + 1, :],
                              in_=dummy_h[:, :])
            nc.sync.dma_start(out=ag_own[OWN:OWN + 1, :], in_=dummy_h[:, :])

            groups_pair = [[2 * k, 2 * k + 1] for k in range(4)]
            groups_quad = [[0, 2, 4, 6], [1, 3, 5, 7]]

            # mask prefetch bookkeeping: (pass_key, ci) -> smT tile. Only
            # smT lives in DRAM; sm is derived on-chip by PE transpose in
            # the front->back slack of the chunk pipeline.
            premask = {}

            def load_masks(key, sched, smT_in, ci):
                if (key, ci) in premask:
                    return premask.pop((key, ci))
                (_t0, _nt, m0, nm) = sched["chunks"][ci]
                smT = mkp.tile([128, NM_MAX * 128], bf16, tag="smT")
                nc.sync.dma_start(
                    out=smT[:, 0:nm * 128],
                    in_=smT_in[:, m0 * 128:(m0 + nm) * 128])
                return smT

            def prefetch_masks(key, sched, smT_in, n=2):
                for ci in range(min(n, len(sched["chunks"]))):
                    premask[(key, ci)] = load_masks(key + "_", sched,
                                                    smT_in, ci)

            # er_sb: per-layer er rows in partial-permuted block order, one
            # tile per range so readers only depend on their own range's
            # split er AllGather. Double buffered by layer parity: the next
            # layer's er loads land while the current layer's edge passes
            # still read the old values.
            er_sb = [[pp.tile([128, (p_hi - p_lo) // 128, 4], bf16,
                              name=f"er_sb{par}_{ri}")
                      for ri, (_b, p_lo, p_hi, _o, _o2) in
                      enumerate(RS_PLAN)]
                     for par in range(2)]

            def load_er_range(li, r):
                (_bl, p_lo, p_hi, _o_lo, _o_hi) = RS_PLAN[r]
                nc.sync.dma_start(
                    out=er_sb[li % 2][r][:],
                    in_=er_tab[p_lo:p_hi, :]
                        .rearrange("(t p) c -> p t c", p=128))

            def proj_range(li, src_hT, r, prange_pool, prj_ps):
                """Project own blocks of OWN_RANGES[r]: write ag_own rows,
                ag_er rows, then fire the split pair-AllGather of er."""
                (o_lo, o_hi) = OWN_RANGES[r]
                for b0 in range(o_lo, o_hi, 8):
                    nb = min(8, o_hi - b0)
                    tabrow = prange_pool.tile([128, 8, TCOLS], bf16,
                                              tag="tabrow")
                    errow = prange_pool.tile([128, 8, 4], bf16, tag="errow")
                    for t in range(nb):
                        pj = prj_ps.tile([128, 136], f32, space="PSUM",
                                         tag="aux")
                        nc.tensor.matmul(
                            pj[:],
                            lhsT=src_hT[:, (b0 + t) * 128:(b0 + t + 1) * 128],
                            rhs=waug_sb[li][:, 0:136], start=True, stop=True)
                        nc.scalar.activation(tabrow[:, t, 0:132],
                                             pj[:, 0:132], AF.Copy)
                        nc.scalar.activation(errow[:, t, :], pj[:, 132:136],
                                             AF.Copy)
                    nc.sync.dma_start(
                        out=ag_own[b0 * 128:(b0 + nb) * 128, :]
                            .rearrange("(t p) c -> p t c", p=128),
                        in_=tabrow[:, 0:nb, :])
                    nc.sync.dma_start(
                        out=ag_pack[b0 * 128:(b0 + nb) * 128, :]
                            .rearrange("(t p) c -> p t c", p=128),
                        in_=tabrow[:, 0:nb, 0:PCOLS])
                    nc.sync.dma_start(
                        out=ag_er[b0 * 128:(b0 + nb) * 128, :]
                            .rearrange("(t p) c -> p t c", p=128),
                        in_=errow[:, 0:nb, :])
                (_bl, p_lo, p_hi, ro_lo, ro_hi) = RS_PLAN[r]
                if no_cc:
                    sz = ro_hi - ro_lo
                    nc.sync.dma_start(out=er_tab[p_lo:p_lo + sz, :],
                                      in_=ag_er[ro_lo:ro_hi, :])
                    nc.sync.dma_start(out=er_tab[p_lo + sz:p_hi, :],
                                      in_=ag_er[ro_lo:ro_hi, :])
                else:
                    nc.gpsimd.collective_compute(
                        "AllGather", mybir.AluOpType.bypass,
                        replica_groups=groups_pair,
                        ins=[ag_er[ro_lo:ro_hi, :]],
                        outs=[er_tab[p_lo:p_hi, :]])
                load_er_range(li, r)

            def table_ag_piece(li, r):
                """Quad-AllGather the packed own rows of range r into the
                contiguous packed table."""
                (o_lo, o_hi) = OWN_RANGES[r]
                sz = (o_hi - o_lo) * 128
                t0 = T_PBASE[r]
                if no_cc:
                    for rep in range(4):
                        nc.sync.dma_start(
                            out=table_pack[t0 + rep * sz:t0 + (rep + 1) * sz,
                                           :],
                            in_=ag_pack[o_lo * 128:o_hi * 128, :])
                else:
                    nc.gpsimd.collective_compute(
                        "AllGather", mybir.AluOpType.bypass,
                        replica_groups=groups_quad,
                        ins=[ag_pack[o_lo * 128:o_hi * 128, :]],
                        outs=[table_pack[t0:t0 + 4 * sz, :]])

            def restripe_piece(li, r):
                # 264B-packed -> 512B-stride expansion; emit only at points
                # where the matching AG piece is already complete, else the
                # in-order sync DMA queue bubbles behind the wait.
                (o_lo, o_hi) = OWN_RANGES[r]
                sz = (o_hi - o_lo) * 128
                t0 = T_PBASE[r]
                nc.sync.dma_start(
                    out=table[li % 2][t0:t0 + 4 * sz, 0:PCOLS],
                    in_=table_pack[t0:t0 + 4 * sz, :])

            def post_range(li, r, prange_pool, prj_ps):
                """Divide/bias/activation for own blocks of OWN_RANGES[r];
                for layers 0/1 follow with the next layer's projection of
                the same rows, for the last layer write y output rows."""
                last = li == 2
                (o_lo, o_hi) = OWN_RANGES[r]
                dst_hT = hT2 if li % 2 == 0 else hT
                for b0 in range(o_lo, o_hi, 8):
                    nb = min(8, o_hi - b0)
                    osum = prange_pool.tile([128, 8, 132], f32, tag="osum")
                    nc.sync.dma_start(
                        out=osum[:, 0:nb, :],
                        in_=own_sum[b0 * 128:(b0 + nb) * 128, :]
                            .rearrange("(t p) c -> p t c", p=128))
                    den = prange_pool.tile([128, 8, 4], f32, tag="den")
                    nc.vector.tensor_scalar_max(den[:, 0:nb, :],
                                                osum[:, 0:nb, 128:132], EPS)
                    rec = prange_pool.tile([128, 8, 4, 1], f32, tag="rec")
                    nc.vector.reciprocal(rec[:, 0:nb, :, 0], den[:, 0:nb, :])
                    if not last:
                        o2 = prange_pool.tile([128, 8, 128], f32, tag="o2")
                        nc.vector.tensor_tensor(
                            out=o2[:, 0:nb, :]
                                .rearrange("p t (h d) -> p t h d", h=4),
                            in0=osum[:, 0:nb, 0:128]
                                .rearrange("p t (h d) -> p t h d", h=4),
                            in1=rec[:, 0:nb, :, :]
                                .to_broadcast([128, nb, 4, 32]),
                            op=OP.mult)
                        nc.vector.tensor_tensor(
                            out=o2[:, 0:nb, :], in0=o2[:, 0:nb, :],
                            in1=brep_sb[li][:]
                                .rearrange("p (t c) -> p t c", t=1)
                                .to_broadcast([128, nb, 128]),
                            op=OP.add)
                        # ELU via scalar engine: exn = exp(-relu(-x)) =
                        # exp(min(x,0)); o2 = relu(x) + exn - 1
                        exn = prange_pool.tile([128, 8, 128], f32, tag="exn")
                        nc.scalar.activation(exn[:, 0:nb, :], o2[:, 0:nb, :],
                                             AF.Relu, scale=-1.0)
                        nc.scalar.activation(exn[:, 0:nb, :], exn[:, 0:nb, :],
                                             AF.Exp, scale=-1.0)
                        nc.scalar.activation(o2[:, 0:nb, :], o2[:, 0:nb, :],
                                             AF.Relu)
                        nc.vector.tensor_tensor(out=o2[:, 0:nb, :],
                                                in0=o2[:, 0:nb, :],
                                                in1=exn[:, 0:nb, :],
                                                op=OP.add)
                        nc.scalar.activation(o2[:, 0:nb, :], o2[:, 0:nb, :],
                                             AF.Copy, bias=-1.0)
                        for t in range(nb):
                            tp = prj_ps.tile([128, 136], f32, space="PSUM",
                                             tag="aux")
                            nc.tensor.matmul(tp[:, 0:128], lhsT=o2[:, t, :],
                                             rhs=ident_sb[:], start=True,
                                             stop=True)
                            nc.scalar.activation(
                                dst_hT[:, (b0 + t) * 128:(b0 + t + 1) * 128],
                                tp[:, 0:128], AF.Copy)
                    else:
                        r4 = prange_pool.tile([128, 8, 4, 32], f32, tag="r4")
                        nc.vector.tensor_tensor(
                            out=r4[:, 0:nb, :, :],
                            in0=osum[:, 0:nb, 0:128]
                                .rearrange("p t (h d) -> p t h d", h=4),
                            in1=rec[:, 0:nb, :, :]
                                .to_broadcast([128, nb, 4, 32]),
                            op=OP.mult)
                        r1 = prange_pool.tile([128, 8, 32], f32, tag="r1")
                        nc.vector.tensor_tensor(out=r1[:, 0:nb, :],
                                                in0=r4[:, 0:nb, 0, :],
                                                in1=r4[:, 0:nb, 1, :],
                                                op=OP.add)
                        r2 = prange_pool.tile([128, 8, 32], f32, tag="r2")
                        nc.vector.tensor_tensor(out=r2[:, 0:nb, :],
                                                in0=r4[:, 0:nb, 2, :],
                                                in1=r4[:, 0:nb, 3, :],
                                                op=OP.add)
                        nc.vector.tensor_tensor(out=r1[:, 0:nb, :],
                                                in0=r1[:, 0:nb, :],
                                                in1=r2[:, 0:nb, :], op=OP.add)
                        nc.vector.tensor_scalar_mul(r1[:, 0:nb, :],
                                                    r1[:, 0:nb, :], 0.25)
                        nc.vector.tensor_tensor(
                            out=r1[:, 0:nb, :], in0=r1[:, 0:nb, :],
                            in1=brep_sb[li][:]
                                .rearrange("p (t c) -> p t c", t=1)
                                .to_broadcast([128, nb, 32]),
                            op=OP.add)
                        nfull = nb if (b0 + nb) * 128 <= NPC else nb - 1
                        if nfull > 0:
                            nc.sync.dma_start(
                                out=y_out[b0 * 128:(b0 + nfull) * 128, :]
                                    .rearrange("(t p) c -> p t c", p=128),
                                in_=r1[:, 0:nfull, :])
                        if nfull < nb:
                            rem = NPC - (b0 + nfull) * 128
                            nc.sync.dma_start(
                                out=y_out[(b0 + nfull) * 128:NPC, :],
                                in_=r1[0:rem, nfull, :])
                if not last:
                    proj_range(li + 1, dst_hT, r, prange_pool, prj_ps)
                    # table AG pieces: r=0,1 fire mid-pass-B (CC has slack
                    # there); r=2,3 are deferred past the RS pieces so they
                    # never delay the boundary-critical ReduceScatters.
                    if r <= 1:
                        table_ag_piece(li + 1, r)
                    elif r == 3:
                        table_ag_piece(li + 1, 2)
                        table_ag_piece(li + 1, 3)
                        # AG pieces 0/1 fired mid-pass-B and are done;
                        # restripe them now. Pieces 2/3 restripe during the
                        # next layer's pass A (front_hook).
                        restripe_piece(li + 1, 0)
                        restripe_piece(li + 1, 1)

            # ---- layer 0 init: load xT + streamed projection ----
            prefetch_masks("A0", schedA, smTA_in)
            with tc.tile_pool(name="prj0", bufs=2) as p0p, \
                 tc.tile_pool(name="prj0ps", bufs=3, space="PSUM") as p0ps:
                for r, (o_lo, o_hi) in enumerate(OWN_RANGES):
                    nc.sync.dma_start(
                        out=hT[:, o_lo * 128:o_hi * 128],
                        in_=xT_in[:, o_lo * 128:o_hi * 128])
                    proj_range(0, hT, r, p0p, p0ps)
                    table_ag_piece(0, r)
                for r in range(len(OWN_RANGES)):
                    restripe_piece(0, r)

            for li in range(3):
                last = li == 2

                with tc.tile_pool(name=f"gt{li}", bufs=6) as gp, \
                     tc.tile_pool(name=f"ms{li}", bufs=4) as mp, \
                     tc.tile_pool(name=f"ex{li}", bufs=4) as xp, \
                     tc.tile_pool(name=f"pb{li}", bufs=4) as pbp, \
                     tc.tile_pool(name=f"pa{li}", bufs=4) as pap, \
                     tc.tile_pool(name=f"sg{li}", bufs=2,
                                  space="PSUM") as sgps, \
                     tc.tile_pool(name=f"aux{li}", bufs=3,
                                  space="PSUM") as auxps, \
                     tc.tile_pool(name=f"er{li}", bufs=3,
                                  space="PSUM") as erps:

                    def edge_pass(sched, idx_sb, smT_in, tab, passB,
                                  mkey, on_back_done=None, front_hook=None):
                        seg_tiles = {}
                        pa_tiles = {}
                        rs_left = [len(bl) for (bl, *_r) in RS_PLAN]
                        rs_fired_at = [None] * len(RS_PLAN)
                        state = {}

                        def emit_front(ci):
                            # gather + mask stream + er matmuls for chunk ci;
                            # then derive sm = transpose(smT) on the PE for
                            # the accumulate matmuls two chunks later.
                            (t0, nt, m0, nm) = sched["chunks"][ci]
                            g = gp.tile([128, CHUNK, TCOLS], bf16, tag="g")
                            nc.gpsimd.dma_gather(
                                out_ap=g[:, 0:nt, :], in_ap=tab[:, :],
                                idxs_ap=idx_sb[:, t0 * 8:(t0 + nt) * 8],
                                num_idxs=nt * 128, num_idxs_reg=nt * 128,
                                elem_size=TCOLS, single_packet=False)
                            smT = load_masks(mkey, sched, smT_in, ci)
                            er_ps = erps.tile([128, CHUNK, 4], f32,
                                              space="PSUM", tag="er_ps")
                            for t in range(nt):
                                for (ms, b, ef, el_, _sf, _sl) in \
                                        sched["tiles"][t0 + t]:
                                    lm = ms - m0
                                    nc.tensor.matmul(
                                        er_ps[:, t, :],
                                        lhsT=smT[:, lm * 128:(lm + 1) * 128],
                                        rhs=er_sb[li % 2][int(RNG_OF[b])][
                                            :, int(LROW_OF[b]), :],
                                        start=ef, stop=el_)
                            sm = mkp.tile([128, NM_MAX * 128], bf16, tag="sm")
                            for lm in range(nm):
                                st = auxps.tile([128, 136], f32,
                                                space="PSUM", tag="aux")
                                nc.tensor.matmul(
                                    st[:, 0:128],
                                    lhsT=smT[:, lm * 128:(lm + 1) * 128],
                                    rhs=identb_sb[:], start=True, stop=True)
                                nc.scalar.activation(
                                    sm[:, lm * 128:(lm + 1) * 128],
                                    st[:, 0:128], AF.Copy)
                            state[ci] = (t0, nt, m0, g, sm, er_ps)

                        def emit_back(ci):
                            (t0, nt, m0, g, sm, er_ps) = state.pop(ci)
                            for g0 in range(0, nt, GROUP):
                                gl = min(GROUP, nt - g0)
                                e4 = xp.tile([128, GROUP, 4], f32, tag="e4")
                                nc.vector.tensor_tensor(
                                    out=e4[:, 0:gl, :],
                                    in0=g[:, g0:g0 + gl, 128:132],
                                    in1=er_ps[:, g0:g0 + gl, :], op=OP.add)
                                lr = xp.tile([128, GROUP, 4], f32, tag="lr")
                                nc.scalar.activation(lr[:, 0:gl, :],
                                                     e4[:, 0:gl, :],
                                                     AF.Prelu, alpha=NEG)
                                ex4 = xp.tile([128, GROUP, 4, 1], f32,
                                              tag="ex4")
                                nc.scalar.activation(ex4[:, 0:gl, :, 0],
                                                     lr[:, 0:gl, :], AF.Exp)
                                m4 = mp.tile([128, GROUP, 132], bf16, tag="m4")
                                nc.scalar.activation(m4[:, 0:gl, 128:132],
                                                     ex4[:, 0:gl, :, 0],
                                                     AF.Copy)
                                nc.vector.tensor_tensor(
                                    out=m4[:, 0:gl, 0:128],
                                    in0=g[:, g0:g0 + gl, 0:128],
                                    in1=ex4[:, 0:gl, :, :]
                                        .to_broadcast([128, gl, 4, 32]),
                                    op=OP.mult)
                                for t in range(gl):
                                    for (ms, b, _ef, _el, sf, sl) in \
                                            sched["tiles"][t0 + g0 + t]:
                                        lm = ms - m0
                                        if sf:
                                            seg_tiles[b] = sgps.tile(
                                                [128, 132], f32, space="PSUM",
                                                tag="seg",
                                                name=f"seg{li}_{passB}_{b}")
                                            if passB:
                                                pa = pap.tile([128, 132], f32,
                                                              tag="pa",
                                                              name=f"pa{li}_{b}")
                                                ri_ = int(RNG_OF[b])
                                                lr = int(LROW_OF[b])
                                                nc.sync.dma_start(
                                                    out=pa[:],
                                                    in_=partialA[ri_][
                                                        lr * 128:
                                                        (lr + 1) * 128, :])
                                                pa_tiles[b] = pa
                                        nc.tensor.matmul(
                                            seg_tiles[b][:],
                                            lhsT=sm[:, lm * 128:(lm + 1) * 128],
                                            rhs=m4[:, t, :],
                                            start=sf, stop=sl)
                                        if sl:
                                            pb = pbp.tile([128, 132], f32,
                                                          tag="pb")
                                            ri_ = int(RNG_OF[b])
                                            lr = int(LROW_OF[b])
                                            if passB:
                                                nc.vector.tensor_tensor(
                                                    out=pb[:],
                                                    in0=seg_tiles[b][:],
                                                    in1=pa_tiles.pop(b)[:],
                                                    op=OP.add)
                                                nc.sync.dma_start(
                                                    out=partial[ri_][
                                                        lr * 128:
                                                        (lr + 1) * 128, :],
                                                    in_=pb[:])
                                                (bl, p_lo, p_hi, so_lo,
                                                 so_hi) = RS_PLAN[ri_]
                                                rs_left[ri_] -= 1
                                                if rs_left[ri_] == 0:
                                                    if not no_cc:
                                                        nc.gpsimd.\
                                                            collective_compute(
                                                            "ReduceScatter",
                                                            mybir.AluOpType.add,
                                                            replica_groups=
                                                            groups_pair,
                                                            ins=[partial[ri_]
                                                                 [:, :]],
                                                            outs=[own_sum[
                                                                so_lo:so_hi,
                                                                :]])
                                                    else:
                                                        sz = so_hi - so_lo
                                                        nc.sync.dma_start(
                                                            out=own_sum[
                                                                so_lo:so_hi,
                                                                :],
                                                            in_=partial[ri_][
                                                                0:sz, :])
                                                    rs_fired_at[ri_] = ci
                                            else:
                                                nc.scalar.activation(
                                                    pb[:], seg_tiles[b][:],
                                                    AF.Copy)
                                                nc.sync.dma_start(
                                                    out=partialA[ri_][
                                                        lr * 128:
                                                        (lr + 1) * 128, :],
                                                    in_=pb[:])
                                            seg_tiles.pop(b)
                            if on_back_done is not None:
                                on_back_done(ci, rs_fired_at)

                        nchunk = len(sched["chunks"])
                        AHEAD = 2
                        for ci in range(min(AHEAD, nchunk)):
                            emit_front(ci)
                            if front_hook is not None:
                                front_hook(ci)
                        for ci in range(AHEAD, nchunk):
                            emit_front(ci)
                            if front_hook is not None:
                                front_hook(ci)
                            emit_back(ci - AHEAD)
                        for ci in range(max(0, nchunk - AHEAD), nchunk):
                            emit_back(ci)

                    # post/proj streaming behind the split RS pieces.
                    # RS_DELAY chunks of slack before emitting each range's
                    # post ops so engine queues never stall on the RS sem.
                    RS_DELAY = [3, 2, 2, 0]
                    posted = [False] * len(RS_PLAN)
                    prefA_done = [False]

                    with tc.tile_pool(name=f"pr{li}", bufs=2) as prp:
                        prps = auxps

                        def on_back_done(ci, rs_fired_at):
                            nchunk = len(schedB["chunks"])
                            # prefetch next layer's pass-A masks mid-pass-B
                            if not last and not prefA_done[0] and \
                                    ci >= nchunk // 2:
                                prefetch_masks(f"A{li + 1}", schedA, smTA_in)
                                prefA_done[0] = True
                            for ri in range(len(RS_PLAN)):
                                if posted[ri] or rs_fired_at[ri] is None:
                                    continue
                                ready = (rs_fired_at[ri] + RS_DELAY[ri] <= ci
                                         or ci == nchunk - 1)
                                if ready and (ri == 0 or posted[ri - 1]):
                                    post_range(li, ri, prp, prps)
                                    posted[ri] = True

                        def front_hook_A(ci):
                            if li > 0 and ci == 5:
                                restripe_piece(li, 2)
                            elif li > 0 and ci == 7:
                                restripe_piece(li, 3)

                        edge_pass(schedA, idxA_sb, smTA_in, ag_own,
                                  False, f"A{li}", front_hook=front_hook_A)
                        prefetch_masks(f"B{li}", schedB, smTB_in)
                        edge_pass(schedB, idxB_sb, smTB_in,
                                  table[li % 2], True, f"B{li}", on_back_done)
                        for ri in range(len(RS_PLAN)):
                            if not posted[ri]:
                                post_range(li, ri, prp, prps)
                                posted[ri] = True
    nc.finalize()
    return nc


def _make_consts(W0, al0, ar0, b0, W1, al1, ar1, b1, W2, al2, ar2, b2):
    consts = {}
    for li, (W, al, ar, b) in enumerate(
            [(W0, al0, ar0, b0), (W1, al1, ar1, b1), (W2, al2, ar2, b2)]):
        consts[f"Waug{li}"] = _augment(np.asarray(W, np.float32),
                                       np.asarray(al, np.float32),
                                       np.asarray(ar, np.float32)).astype(
                                           ml_dtypes.bfloat16)
        b = np.asarray(b, np.float32)
        if li < 2:
            consts[f"brep{li}"] = np.tile(b.reshape(1, 128), (128, 1))
        else:
            consts[f"brep{li}"] = np.tile(b.reshape(H, D).mean(0).reshape(1, D),
                                          (128, 1))
    dummy = np.zeros((1, TCOLS), ml_dtypes.bfloat16)
    dummy[0, 128:132] = ml_dtypes.bfloat16(-1e30)
    consts["dummyrow"] = dummy
    return consts


def _in_maps(x):
    cores = _cache["pre"][0]
    x = np.asarray(x, dtype=np.float32)
    in_maps = []
    for c in range(NCORE):
        lo, hi = _own_rows(c)
        xT = np.zeros((128, OWN), ml_dtypes.bfloat16)
        xT[:, 0:NPC] = x[lo:hi].T.astype(ml_dtypes.bfloat16)
        cc = cores[c]
        in_maps.append(dict(xT=xT, idxA=cc["idxA"], idxB=cc["idxB"],
                            smTA=np.asarray(cc["smTA"]),
                            smTB=np.asarray(cc["smTB"])))
    return in_maps


def kernel(x, src, dst, W0, al0, ar0, b0, W1, al1, ar1, b1, W2, al2, ar2, b2):
    from concourse.bass_utils import run_bass_kernel_spmd

    key = (hash(np.asarray(src).tobytes()) ^ hash(np.asarray(dst).tobytes()))
    if "pre" not in _cache or _cache.get("prekey") != key:
        _cache["pre"] = _preprocess(src, dst)
        _cache["prekey"] = key
    cores, schedA, schedB = _cache["pre"]

    consts = _make_consts(W0, al0, ar0, b0, W1, al1, ar1, b1, W2, al2, ar2, b2)

    ck = key ^ hash(consts["Waug0"].tobytes())
    if "nc" not in _cache or _cache.get("nckey") != ck:
        _cache["nc"] = _build(schedA, schedB, consts)
        _cache["nckey"] = ck
    nc = _cache["nc"]

    in_maps = _in_maps(x)
    r = run_bass_kernel_spmd(nc, in_maps, list(range(NCORE)))
    y = np.zeros((N, D), np.float32)
    for c in range(NCORE):
        lo, hi = _own_rows(c)
        y[lo:hi] = r.results[c]["y"]
    return y


# revision 73
# speedup vs baseline: 1.1112x; 1.0199x over previous
"""3-layer GAT on 8 Trainium2 NeuronCores (Bass/Tile).

Sharding: 2D graph partition. Pair q = cores {2q, 2q+1} aggregates the dst
nodes of strips [q*6250,(q+1)*6250) and [25000+q*6250, 25000+(q+1)*6250);
even cores take edges with src < 25000, odd cores the rest. Node ownership:
core 2k owns rows [k*6250,(k+1)*6250), core 2k+1 owns [25000+k*6250, ...).

Per layer: each core projects its own rows (feat|el|er via an augmented
weight matrix) into a local gather table, then runs the edge phase in two
passes: pass A covers edges whose source is one of the core's own rows and
gathers from the local table while the quad AllGather of the full src-half
table is still in flight; pass B covers the remaining edges and gathers
from the AllGathered table. Edges are packed into 128-wide tiles grouped
by pairs of 128-dst blocks (a tile may straddle the two blocks; the
host-precomputed one-hot masks select membership). Per-edge er comes from
a transposed one-hot matmul against SBUF-resident per-block er rows (no
second gather). Messages are accumulated per dst block by one-hot-mask
matmuls into PSUM; pass B adds pass A's partial sums back in.

The dst pair-groups are processed in 4 segments; the pairwise
ReduceScatter of partial sums is split into 4 matching pieces (the
partial-row permutation interleaves [A-range | B-range] per piece so each
piece is contiguous), each fired as soon as its blocks complete mid-pass-B.
Post-processing (divide/bias/ELU, head-mean on the last layer) and the
next layer's projection are streamed per own-block range behind each RS
piece, so almost the whole layer boundary hides under pass B. The er
AllGather is split per range the same way (er_tab shares the partial-row
permutation, keeping each piece contiguous); mask loads for the first
chunks of each pass are prefetched during the previous pass. Gather calls
carry trailing -1 indices so the Q7 descriptor generator truncates padded
tail slots.
"""

import numpy as np
import ml_dtypes

N = 50000
E = 800000
F = 128                  # input feats and hidden width (4 heads x 32)
H = 4
D = 32
NEG = 0.2
NCORE = 8
NPC = 6250               # nodes owned per core
OWN = 6272               # 49*128, padded own rows
OWNBLK = 49
PAIR = 12544             # 98*128 dst slots per pair
NBLK = 98
NGRP = 49                # pair-groups of 2 blocks
HALF = 25088             # 4*OWN rows per src-half table
TROWS = 25216            # HALF + 128 (dummy row at HALF)
DUMMY = HALF
TCOLS = 256              # bf16 cols: feat(128) | el(4) | pad
PCOLS = 132              # packed AllGather row: feat(128) | el(4)
CHUNK = 20               # max tiles per dma_gather call
GROUP = 8                # tiles per vector-op batch
EPS = 1e-30

# Own-block ranges: post/proj are streamed per range, each behind its own
# ReduceScatter piece. The last range is small so the layer-boundary chain
# (last RS piece -> post -> proj -> table write) is short.
OWN_RANGES = [(0, 22), (22, 34), (34, 45), (45, 49)]

# pass-B pair-group order: 4 segments, segment r completes the blocks of
# OWN_RANGES[r] (A side: blocks lo..hi-1, B side: 49+lo..49+hi-1), so each
# ReduceScatter piece fires as early as possible. Straddle groups whose
# later-range block completes early are harmless (counters are per block).
GORDER_B = (list(range(0, 11)) + list(range(24, 36)) +    # seg 1
            list(range(11, 17)) + list(range(36, 42)) +   # seg 2
            list(range(17, 23)) + list(range(42, 47)) +   # seg 3
            [23, 47, 48])                                 # seg 4

# pass-A pair-group order: sorted by the er-AllGather piece each group
# needs (max over its two blocks), so groups needing the late pieces sit at
# the end of pass A and never stall the in-order engine queues.
GORDER_A = (list(range(0, 11)) + list(range(25, 35)) +    # er range 1
            list(range(11, 17)) + list(range(35, 41)) +   # er range 2
            list(range(17, 22)) + list(range(41, 47)) +   # er range 3
            [22, 23, 24, 47, 48])                         # er range 4

# block -> partial-row-block permutation: [A1 B1 | A2 B2 | A3 B3 | A4 B4]
# where Ar/Br are the A/B-side blocks of OWN_RANGES[r]. The same layout is
# used for er_tab so the split pair-AllGather outputs stay contiguous.
ROW_OF = np.empty(NBLK, np.int64)
RNG_OF = np.empty(NBLK, np.int64)    # block -> range index
LROW_OF = np.empty(NBLK, np.int64)   # block -> row-block within its range
RS_PLAN = []             # (blockset, p_lo, p_hi, o_lo, o_hi) per range
_base = 0
for _ri, (_lo, _hi) in enumerate(OWN_RANGES):
    _sz = _hi - _lo
    ROW_OF[_lo:_hi] = _base + np.arange(_sz)
    ROW_OF[49 + _lo:49 + _hi] = _base + _sz + np.arange(_sz)
    RNG_OF[_lo:_hi] = _ri
    RNG_OF[49 + _lo:49 + _hi] = _ri
    LROW_OF[_lo:_hi] = np.arange(_sz)
    LROW_OF[49 + _lo:49 + _hi] = _sz + np.arange(_sz)
    _blocks = frozenset(range(_lo, _hi)) | frozenset(range(49 + _lo, 49 + _hi))
    RS_PLAN.append((_blocks, _base * 128, (_base + 2 * _sz) * 128,
                    _lo * 128, _hi * 128))
    _base += 2 * _sz
assert _base == NBLK

# Range-major layout of the quad-AllGathered src-half table: range r holds
# its 4 ranks' own-row stripes contiguously, so the table AllGather (and
# the local 264B->512B re-stripe) splits into 4 contiguous pieces.
T_PBASE = []
_tb = 0
for _lo, _hi in OWN_RANGES:
    T_PBASE.append(_tb)
    _tb += 4 * (_hi - _lo) * 128
assert _tb == HALF
# own-local row -> range-major table row offset (within one rank's stripe
# the rows keep own-local order; rank k of range r starts at
# T_PBASE[r] + k * sz_r * 128).
_OROW_RANGE = np.empty(OWN, np.int64)   # own row -> range idx
_OROW_LOCAL = np.empty(OWN, np.int64)   # own row -> row within range stripe
for _ri, (_lo, _hi) in enumerate(OWN_RANGES):
    _OROW_RANGE[_lo * 128:_hi * 128] = _ri
    _OROW_LOCAL[_lo * 128:_hi * 128] = np.arange((_hi - _lo) * 128)

_cache = {}


def _schedule(cnt, gorder):
    """Core-uniform tile/mask schedule for one pass.

    cnt: [NCORE, NBLK] per-core per-block edge counts.
    Returns dict with T, nmask, chunks, tiles (per tile: list of
    (mslot, block, er_first, er_last, sc_first, sc_last)).
    """
    GORDER = gorder
    n0 = cnt[:, 0::2]                      # [NCORE, NGRP]
    n1 = cnt[:, 1::2]
    TP = np.maximum(1, np.ceil((n0 + n1).max(axis=0) / 128).astype(np.int64))

    base_tile = {}
    acc = 0
    for g in GORDER:
        base_tile[g] = acc
        acc += int(TP[g])
    T = acc

    tiles = []            # per tile: list of [mslot, block]
    tile_group = []
    # which (tile-in-group, block-parity) pairs are needed on any core; ensure
    # every block gets at least one occurrence (tile 0 fallback)
    need = {}
    for g in GORDER:
        for i in range(int(TP[g])):
            need[(g, i, 0)] = bool((n0[:, g] > 128 * i).any())
            need[(g, i, 1)] = bool(
                ((n0[:, g] < 128 * (i + 1)) &
                 (n0[:, g] + n1[:, g] > 128 * i)).any())
        if not any(need[(g, i, 0)] for i in range(int(TP[g]))):
            need[(g, 0, 0)] = True
        if not any(need[(g, i, 1)] for i in range(int(TP[g]))):
            need[(g, 0, 1)] = True
    mslot = 0
    for g in GORDER:
        for i in range(int(TP[g])):
            ml = []
            if need[(g, i, 0)]:
                ml.append([mslot, 2 * g])
                mslot += 1
            if need[(g, i, 1)]:
                ml.append([mslot, 2 * g + 1])
                mslot += 1
            assert ml
            tiles.append(ml)
            tile_group.append(g)
    nmask = mslot

    # per-block first/last occurrence
    occ = {}
    for ti, ml in enumerate(tiles):
        for m in ml:
            occ.setdefault(m[1], []).append((ti, m[0]))
    first = {b: o[0] for b, o in occ.items()}
    last = {b: o[-1] for b, o in occ.items()}
    sched_tiles = []
    for ti, ml in enumerate(tiles):
        entry = []
        for k, (ms, b) in enumerate(ml):
            entry.append((ms, b,
                          k == 0, k == len(ml) - 1,
                          first[b] == (ti, ms), last[b] == (ti, ms)))
        sched_tiles.append(entry)

    # chunks aligned to pair-group boundaries, up to CHUNK tiles
    chunks = []
    t0 = 0
    ti = 0
    for g in GORDER:
        ti += int(TP[g])
        nxt = None
        gi = GORDER.index(g)
        if gi + 1 < len(GORDER):
            nxt = int(TP[GORDER[gi + 1]])
        if nxt is None or ti - t0 + nxt > CHUNK:
            m0 = min(m[0] for m in sched_tiles[t0]) if sched_tiles[t0] else 0
            mend = max(m[0] for m in sched_tiles[ti - 1]) + 1
            chunks.append((t0, ti - t0, m0, mend - m0))
            t0 = ti
    assert t0 == T
    return dict(T=T, nmask=nmask, chunks=chunks, tiles=sched_tiles,
                tile_group=tile_group, base_tile=base_tile, TP=TP,
                gorder=gorder)


def _wrap16(a):
    # value i of each 128-group at [i%16, i//16], replicated per 16 rows
    t = a.reshape(-1, 128)                     # [T, 128]
    w = t.reshape(t.shape[0], 8, 16)           # [T, 8, 16]
    w = w.transpose(2, 0, 1).reshape(16, -1)   # [16, T*8]
    return np.tile(w, (8, 1)).astype(np.int16)  # [128, T*8]


def _core_pass_arrays(sched, rloc_e, rows_e, pad_row, trunc_from_chunk):
    """Build idx + mask streams for one (core, pass).

    rloc_e: pair-local dst row per edge; rows_e: gather-table row per edge.
    Chunks with index >= trunc_from_chunk get their trailing padded idx
    slots set to -1 (the Q7 truncates them); earlier chunks keep the dummy
    row so first-touch SBUF tiles never expose uninitialized data.
    """
    T, nmask = sched["T"], sched["nmask"]
    base_tile = sched["base_tile"]
    GORDER = sched["gorder"]
    # group rank of each edge
    grank_of = np.empty(NGRP, np.int64)
    for r, g in enumerate(GORDER):
        grank_of[g] = r
    pg = rloc_e // 256
    gr = grank_of[pg]
    order = np.lexsort((rloc_e, gr))
    rloc_s = rloc_e[order]
    rows_s = rows_e[order]
    gr_s = gr[order]
    # position within group
    starts = np.searchsorted(gr_s, np.arange(len(GORDER)))
    pos_in_group = np.arange(len(gr_s)) - starts[gr_s]
    base128 = np.array([base_tile[GORDER[r]] * 128
                        for r in range(len(GORDER))], np.int64)
    s_glob = base128[gr_s] + pos_in_group

    idx = np.full(T * 128, pad_row, np.int64)
    idx[s_glob] = rows_s
    real = np.zeros(T * 128, bool)
    real[s_glob] = True
    for ci, (t0, nt, _m0, _nm) in enumerate(sched["chunks"]):
        if ci < trunc_from_chunk:
            continue
        a, b = t0 * 128, (t0 + nt) * 128
        nz = np.flatnonzero(real[a:b])
        last = nz[-1] if len(nz) else -1
        idx[a + last + 1:b] = -1

    # mask slot lookup per (tile, block-parity)
    mslot_of = np.full((T, 2), -1, np.int64)
    for ti, ml in enumerate(sched["tiles"]):
        g = sched["tile_group"][ti]
        for (ms, b, *_fl) in ml:
            mslot_of[ti, b - 2 * g] = ms
    ti_e = s_glob // 128
    e_e = s_glob % 128
    b_e = rloc_s // 128
    s128_e = rloc_s % 128
    par = b_e - 2 * np.array(sched["tile_group"])[ti_e]
    ms_e = mslot_of[ti_e, par]
    assert (ms_e >= 0).all()

    smatTw = np.zeros((128, nmask * 128), ml_dtypes.bfloat16)
    smatTw[s128_e, ms_e * 128 + e_e] = 1
    return _wrap16(idx), smatTw


def _preprocess(src, dst):
    src = np.asarray(src).astype(np.int64)
    dst = np.asarray(dst).astype(np.int64)
    q = np.where(dst < 25000, dst // NPC, (dst - 25000) // NPC)
    s = (src >= 25000).astype(np.int64)
    core_of = 2 * q + s
    rloc = np.where(dst < 25000, dst - q * NPC, OWN + (dst - 25000 - q * NPC))
    ks = np.where(src < 25000, src // NPC, (src - 25000) // NPC)
    olocal = np.where(src < 25000, src - ks * NPC, (src - 25000) - ks * NPC)
    _ri = _OROW_RANGE[olocal]
    _szr = np.array([(hi - lo) * 128 for lo, hi in OWN_RANGES], np.int64)
    tloc = (np.array(T_PBASE, np.int64)[_ri] + ks * _szr[_ri]
            + _OROW_LOCAL[olocal])
    own = ks == q
    ownrow = np.where(src < 25000, src - ks * NPC, src - 25000 - ks * NPC)
    blk = rloc // 128

    cntA = np.zeros((NCORE, NBLK), np.int64)
    cntB = np.zeros((NCORE, NBLK), np.int64)
    for c in range(NCORE):
        m = core_of == c
        cntA[c] = np.bincount(blk[m & own], minlength=NBLK)
        cntB[c] = np.bincount(blk[m & ~own], minlength=NBLK)
    schedA = _schedule(cntA, GORDER_A)
    schedB = _schedule(cntB, GORDER_B)

    cores = []
    for c in range(NCORE):
        m = core_of == c
        mA = m & own
        mB = m & ~own
        idxA, smTA = _core_pass_arrays(schedA, rloc[mA], ownrow[mA],
                                       OWN, 10**9)
        idxB, smTB = _core_pass_arrays(schedB, rloc[mB], tloc[mB],
                                       DUMMY, 10**9)
        cores.append(dict(idxA=idxA, smTA=smTA, idxB=idxB, smTB=smTB))
    return cores, schedA, schedB


def _own_rows(c):
    k = c // 2
    if c % 2 == 0:
        return k * NPC, (k + 1) * NPC
    return 25000 + k * NPC, 25000 + (k + 1) * NPC


def _augment(W, al, ar):
    dout = W.shape[1] // H
    Wal = np.stack([W[:, h * dout:(h + 1) * dout] @ al[h] for h in range(H)], 1)
    War = np.stack([W[:, h * dout:(h + 1) * dout] @ ar[h] for h in range(H)], 1)
    return np.concatenate([W, Wal, War], 1).astype(np.float32)  # [128, 136]


def _build(schedA, schedB, consts, no_cc=False):
    import concourse.bass as bass
    import concourse.bacc as bacc
    import concourse.tile as tile
    from concourse import mybir
    from concourse.library_config import mlp

    f32 = mybir.dt.float32
    bf16 = mybir.dt.bfloat16
    i16 = mybir.dt.int16
    AF = mybir.ActivationFunctionType
    OP = mybir.AluOpType

    TA, TB = schedA["T"], schedB["T"]
    NMA, NMB = schedA["nmask"], schedB["nmask"]
    NM_MAX = max(max(nm for (_, _, _, nm) in schedA["chunks"]),
                 max(nm for (_, _, _, nm) in schedB["chunks"]))

    nc = bacc.Bacc(num_devices=NCORE)
    xT_in = nc.declare_dram_parameter("xT", [128, OWN], bf16, isOutput=False)
    idxA_in = nc.declare_dram_parameter("idxA", [128, TA * 8], i16,
                                        isOutput=False)
    idxB_in = nc.declare_dram_parameter("idxB", [128, TB * 8], i16,
                                        isOutput=False)
    smTA_in = nc.declare_dram_parameter("smTA", [128, NMA * 128], bf16,
                                        isOutput=False)
    smTB_in = nc.declare_dram_parameter("smTB", [128, NMB * 128], bf16,
                                        isOutput=False)
    y_out = nc.declare_dram_parameter("y", [NPC, D], f32, isOutput=True)

    with tile.TileContext(nc) as tc:
        with tc.tile_pool(name="persist", bufs=1) as pp, \
             tc.tile_pool(name="mk", bufs=3) as mkp, \
             tc.tile_pool(name="dram", bufs=1, space="DRAM") as dp:
            nc.gpsimd.load_library(mlp)

            # ---- persistent SBUF state ----
            idxA_sb = pp.tile([128, TA * 8], i16)
            nc.sync.dma_start(out=idxA_sb[:], in_=idxA_in[:, :])
            idxB_sb = pp.tile([128, TB * 8], i16)
            nc.sync.dma_start(out=idxB_sb[:], in_=idxB_in[:, :])
            hT = pp.tile([128, OWN], bf16)
            hT2 = pp.tile([128, OWN], bf16)

            ident_h = nc.inline_tensor(np.eye(128, dtype=np.float32),
                                       name="ident")
            ident_sb = pp.tile([128, 128], f32)
            nc.sync.dma_start(out=ident_sb[:], in_=ident_h[:, :])
            identb_h = nc.inline_tensor(np.eye(128).astype(ml_dtypes.bfloat16),
                                        name="identb")
            identb_sb = pp.tile([128, 128], bf16)
            nc.sync.dma_start(out=identb_sb[:], in_=identb_h[:, :])

            waug_sb = []
            brep_sb = []
            for li in range(3):
                wh = nc.inline_tensor(consts[f"Waug{li}"], name=f"waug{li}")
                wt = pp.tile([128, 136], bf16, name=f"waug_sb{li}")
                nc.sync.dma_start(out=wt[:], in_=wh[:, :])
                waug_sb.append(wt)
                bh = nc.inline_tensor(consts[f"brep{li}"], name=f"brep{li}")
                bt = pp.tile([128, consts[f"brep{li}"].shape[1]], f32,
                             name=f"brep_sb{li}")
                nc.sync.dma_start(out=bt[:], in_=bh[:, :])
                brep_sb.append(bt)

            dummy_h = nc.inline_tensor(consts["dummyrow"], name="dummyrow")

            # ---- DRAM scratch ----
            # partialA/partial are split per own-block range so pass-B loads
            # of a range only depend on that range's pass-A writes (and the
            # RS pieces only on their own range's writes).
            # The gather table is double buffered by layer parity (the next
            # layer's split AllGather pieces land while the current layer's
            # pass B still gathers from the old table). The AllGather moves
            # packed 264B rows; a local re-stripe expands to the 512B-stride
            # layout the gather needs.
            table = [dp.tile([TROWS, TCOLS], bf16, name=f"table{p}")
                     for p in range(2)]
            table_pack = dp.tile([HALF, PCOLS], bf16)
            er_tab = dp.tile([PAIR, 4], bf16)
            ag_own = dp.tile([OWN + 128, TCOLS], bf16)
            ag_pack = dp.tile([OWN, PCOLS], bf16)
            ag_er = dp.tile([OWN, 4], bf16)
            partial = [dp.tile([(p_hi - p_lo), 132], f32, name=f"partial{ri}")
                       for ri, (_b, p_lo, p_hi, _o, _o2) in
                       enumerate(RS_PLAN)]
            own_sum = dp.tile([OWN, 132], f32)

            nc.sync.dma_start(out=table[0][DUMMY:DUMMY + 1, :],
                              in_=dummy_h[:, :])
            nc.sync.dma_start(out=table[1][DUMMY:DUMMY + 1, :],
                              in_=dummy_h[:, :])
            nc.sync.dma_start(out=ag_own[OWN:OWN + 1, :], in_=dummy_h[:, :])

            groups_pair = [[2 * k, 2 * k + 1] for k in range(4)]
            groups_quad = [[0, 2, 4, 6], [1, 3, 5, 7]]

            # mask prefetch bookkeeping: (pass_key, ci) -> smT tile. Only
            # smT lives in DRAM; sm is derived on-chip by PE transpose in
            # the front->back slack of the chunk pipeline.
            premask = {}

            def load_masks(key, sched, smT_in, ci):
                if (key, ci) in premask:
                    return premask.pop((key, ci))
                (_t0, _nt, m0, nm) = sched["chunks"][ci]
                smT = mkp.tile([128, NM_MAX * 128], bf16, tag="smT")
                nc.sync.dma_start(
                    out=smT[:, 0:nm * 128],
                    in_=smT_in[:, m0 * 128:(m0 + nm) * 128])
                return smT

            def prefetch_masks(key, sched, smT_in, n=2):
                for ci in range(min(n, len(sched["chunks"]))):
                    premask[(key, ci)] = load_masks(key + "_", sched,
                                                    smT_in, ci)

            # pass-A partial sums stay SBUF-resident (bf16): pass B adds
            # them back without any DRAM round-trip.
            paA = pp.tile([128, NBLK, 132], bf16, name="paA")

            # er_sb: per-layer er rows in partial-permuted block order, one
            # tile per range so readers only depend on their own range's
            # split er AllGather. Double buffered by layer parity: the next
            # layer's er loads land while the current layer's edge passes
            # still read the old values.
            er_sb = [[pp.tile([128, (p_hi - p_lo) // 128, 4], bf16,
                              name=f"er_sb{par}_{ri}")
                      for ri, (_b, p_lo, p_hi, _o, _o2) in
                      enumerate(RS_PLAN)]
                     for par in range(2)]

            def load_er_range(li, r):
                (_bl, p_lo, p_hi, _o_lo, _o_hi) = RS_PLAN[r]
                nc.sync.dma_start(
                    out=er_sb[li % 2][r][:],
                    in_=er_tab[p_lo:p_hi, :]
                        .rearrange("(t p) c -> p t c", p=128))

            def proj_range(li, src_hT, r, prange_pool, prj_ps):
                """Project own blocks of OWN_RANGES[r]: write ag_own rows,
                ag_er rows, then fire the split pair-AllGather of er."""
                (o_lo, o_hi) = OWN_RANGES[r]
                for b0 in range(o_lo, o_hi, 6):
                    nb = min(8, o_hi - b0)
                    tabrow = prange_pool.tile([128, 6, TCOLS], bf16,
                                              tag="tabrow")
                    errow = prange_pool.tile([128, 6, 4], bf16, tag="errow")
                    for t in range(nb):
                        pj = prj_ps.tile([128, 136], f32, space="PSUM",
                                         tag="aux")
                        nc.tensor.matmul(
                            pj[:],
                            lhsT=src_hT[:, (b0 + t) * 128:(b0 + t + 1) * 128],
                            rhs=waug_sb[li][:, 0:136], start=True, stop=True)
                        nc.scalar.activation(tabrow[:, t, 0:132],
                                             pj[:, 0:132], AF.Copy)
                        nc.scalar.activation(errow[:, t, :], pj[:, 132:136],
                                             AF.Copy)
                    nc.sync.dma_start(
                        out=ag_own[b0 * 128:(b0 + nb) * 128, :]
                            .rearrange("(t p) c -> p t c", p=128),
                        in_=tabrow[:, 0:nb, :])
                    nc.sync.dma_start(
                        out=ag_pack[b0 * 128:(b0 + nb) * 128, :]
                            .rearrange("(t p) c -> p t c", p=128),
                        in_=tabrow[:, 0:nb, 0:PCOLS])
                    nc.sync.dma_start(
                        out=ag_er[b0 * 128:(b0 + nb) * 128, :]
                            .rearrange("(t p) c -> p t c", p=128),
                        in_=errow[:, 0:nb, :])
                (_bl, p_lo, p_hi, ro_lo, ro_hi) = RS_PLAN[r]
                if no_cc:
                    sz = ro_hi - ro_lo
                    nc.sync.dma_start(out=er_tab[p_lo:p_lo + sz, :],
                                      in_=ag_er[ro_lo:ro_hi, :])
                    nc.sync.dma_start(out=er_tab[p_lo + sz:p_hi, :],
                                      in_=ag_er[ro_lo:ro_hi, :])
                else:
                    nc.gpsimd.collective_compute(
                        "AllGather", mybir.AluOpType.bypass,
                        replica_groups=groups_pair,
                        ins=[ag_er[ro_lo:ro_hi, :]],
                        outs=[er_tab[p_lo:p_hi, :]])
                load_er_range(li, r)

            def table_ag_piece(li, r):
                """Quad-AllGather the packed own rows of range r into the
                contiguous packed table."""
                (o_lo, o_hi) = OWN_RANGES[r]
                sz = (o_hi - o_lo) * 128
                t0 = T_PBASE[r]
                if no_cc:
                    for rep in range(4):
                        nc.sync.dma_start(
                            out=table_pack[t0 + rep * sz:t0 + (rep + 1) * sz,
                                           :],
                            in_=ag_pack[o_lo * 128:o_hi * 128, :])
                else:
                    nc.gpsimd.collective_compute(
                        "AllGather", mybir.AluOpType.bypass,
                        replica_groups=groups_quad,
                        ins=[ag_pack[o_lo * 128:o_hi * 128, :]],
                        outs=[table_pack[t0:t0 + 4 * sz, :]])

            def restripe_piece(li, r):
                # 264B-packed -> 512B-stride expansion; emit only at points
                # where the matching AG piece is already complete, else the
                # in-order sync DMA queue bubbles behind the wait.
                (o_lo, o_hi) = OWN_RANGES[r]
                sz = (o_hi - o_lo) * 128
                t0 = T_PBASE[r]
                nc.sync.dma_start(
                    out=table[li % 2][t0:t0 + 4 * sz, 0:PCOLS],
                    in_=table_pack[t0:t0 + 4 * sz, :])

            def post_range(li, r, prange_pool, prj_ps):
                """Divide/bias/activation for own blocks of OWN_RANGES[r];
                for layers 0/1 follow with the next layer's projection of
                the same rows, for the last layer write y output rows."""
                last = li == 2
                (o_lo, o_hi) = OWN_RANGES[r]
                dst_hT = hT2 if li % 2 == 0 else hT
                for b0 in range(o_lo, o_hi, 6):
                    nb = min(8, o_hi - b0)
                    osum = prange_pool.tile([128, 6, 132], f32, tag="osum")
                    nc.sync.dma_start(
                        out=osum[:, 0:nb, :],
                        in_=own_sum[b0 * 128:(b0 + nb) * 128, :]
                            .rearrange("(t p) c -> p t c", p=128))
                    den = prange_pool.tile([128, 6, 4], f32, tag="den")
                    nc.vector.tensor_scalar_max(den[:, 0:nb, :],
                                                osum[:, 0:nb, 128:132], EPS)
                    rec = prange_pool.tile([128, 6, 4, 1], f32, tag="rec")
                    nc.vector.reciprocal(rec[:, 0:nb, :, 0], den[:, 0:nb, :])
                    if not last:
                        o2 = prange_pool.tile([128, 6, 128], f32, tag="o2")
                        nc.vector.tensor_tensor(
                            out=o2[:, 0:nb, :]
                                .rearrange("p t (h d) -> p t h d", h=4),
                            in0=osum[:, 0:nb, 0:128]
                                .rearrange("p t (h d) -> p t h d", h=4),
                            in1=rec[:, 0:nb, :, :]
                                .to_broadcast([128, nb, 4, 32]),
                            op=OP.mult)
                        nc.vector.tensor_tensor(
                            out=o2[:, 0:nb, :], in0=o2[:, 0:nb, :],
                            in1=brep_sb[li][:]
                                .rearrange("p (t c) -> p t c", t=1)
                                .to_broadcast([128, nb, 128]),
                            op=OP.add)
                        # ELU via scalar engine: exn = exp(-relu(-x)) =
                        # exp(min(x,0)); o2 = relu(x) + exn - 1
                        exn = prange_pool.tile([128, 6, 128], f32, tag="exn")
                        nc.scalar.activation(exn[:, 0:nb, :], o2[:, 0:nb, :],
                                             AF.Relu, scale=-1.0)
                        nc.scalar.activation(exn[:, 0:nb, :], exn[:, 0:nb, :],
                                             AF.Exp, scale=-1.0)
                        nc.scalar.activation(o2[:, 0:nb, :], o2[:, 0:nb, :],
                                             AF.Relu)
                        nc.vector.tensor_tensor(out=o2[:, 0:nb, :],
                                                in0=o2[:, 0:nb, :],
                                                in1=exn[:, 0:nb, :],
                                                op=OP.add)
                        nc.scalar.activation(o2[:, 0:nb, :], o2[:, 0:nb, :],
                                             AF.Copy, bias=-1.0)
                        for t in range(nb):
                            tp = prj_ps.tile([128, 136], f32, space="PSUM",
                                             tag="aux")
                            nc.tensor.matmul(tp[:, 0:128], lhsT=o2[:, t, :],
                                             rhs=ident_sb[:], start=True,
                                             stop=True)
                            nc.scalar.activation(
                                dst_hT[:, (b0 + t) * 128:(b0 + t + 1) * 128],
                                tp[:, 0:128], AF.Copy)
                    else:
                        r4 = prange_pool.tile([128, 6, 4, 32], f32, tag="r4")
                        nc.vector.tensor_tensor(
                            out=r4[:, 0:nb, :, :],
                            in0=osum[:, 0:nb, 0:128]
                                .rearrange("p t (h d) -> p t h d", h=4),
                            in1=rec[:, 0:nb, :, :]
                                .to_broadcast([128, nb, 4, 32]),
                            op=OP.mult)
                        r1 = prange_pool.tile([128, 6, 32], f32, tag="r1")
                        nc.vector.tensor_tensor(out=r1[:, 0:nb, :],
                                                in0=r4[:, 0:nb, 0, :],
                                                in1=r4[:, 0:nb, 1, :],
                                                op=OP.add)
                        r2 = prange_pool.tile([128, 6, 32], f32, tag="r2")
                        nc.vector.tensor_tensor(out=r2[:, 0:nb, :],
                                                in0=r4[:, 0:nb, 2, :],
                                                in1=r4[:, 0:nb, 3, :],
                                                op=OP.add)
                        nc.vector.tensor_tensor(out=r1[:, 0:nb, :],
                                                in0=r1[:, 0:nb, :],
                                                in1=r2[:, 0:nb, :], op=OP.add)
                        nc.vector.tensor_scalar_mul(r1[:, 0:nb, :],
                                                    r1[:, 0:nb, :], 0.25)
                        nc.vector.tensor_tensor(
                            out=r1[:, 0:nb, :], in0=r1[:, 0:nb, :],
                            in1=brep_sb[li][:]
                                .rearrange("p (t c) -> p t c", t=1)
                                .to_broadcast([128, nb, 32]),
                            op=OP.add)
                        nfull = nb if (b0 + nb) * 128 <= NPC else nb - 1
                        if nfull > 0:
                            nc.sync.dma_start(
                                out=y_out[b0 * 128:(b0 + nfull) * 128, :]
                                    .rearrange("(t p) c -> p t c", p=128),
                                in_=r1[:, 0:nfull, :])
                        if nfull < nb:
                            rem = NPC - (b0 + nfull) * 128
                            nc.sync.dma_start(
                                out=y_out[(b0 + nfull) * 128:NPC, :],
                                in_=r1[0:rem, nfull, :])
                if not last:
                    proj_range(li + 1, dst_hT, r, prange_pool, prj_ps)
                    # table AG pieces: r=0,1 fire mid-pass-B (CC has slack
                    # there); r=2,3 are deferred past the RS pieces so they
                    # never delay the boundary-critical ReduceScatters.
                    if r <= 1:
                        table_ag_piece(li + 1, r)
                    elif r == 3:
                        table_ag_piece(li + 1, 2)
                        table_ag_piece(li + 1, 3)
                        # AG pieces 0/1 fired mid-pass-B and are done;
                        # restripe them now. Pieces 2/3 restripe during the
                        # next layer's pass A (front_hook).
                        restripe_piece(li + 1, 0)
                        restripe_piece(li + 1, 1)

            # ---- layer 0 init: load xT + streamed projection ----
            prefetch_masks("A0", schedA, smTA_in)
            with tc.tile_pool(name="prj0", bufs=2) as p0p, \
                 tc.tile_pool(name="prj0ps", bufs=3, space="PSUM") as p0ps:
                for r, (o_lo, o_hi) in enumerate(OWN_RANGES):
                    nc.sync.dma_start(
                        out=hT[:, o_lo * 128:o_hi * 128],
                        in_=xT_in[:, o_lo * 128:o_hi * 128])
                    proj_range(0, hT, r, p0p, p0ps)
                    table_ag_piece(0, r)
                for r in range(len(OWN_RANGES)):
                    restripe_piece(0, r)

            for li in range(3):
                last = li == 2

                with tc.tile_pool(name=f"gt{li}", bufs=6) as gp, \
                     tc.tile_pool(name=f"ms{li}", bufs=4) as mp, \
                     tc.tile_pool(name=f"ex{li}", bufs=4) as xp, \
                     tc.tile_pool(name=f"pb{li}", bufs=4) as pbp, \
                     tc.tile_pool(name=f"sg{li}", bufs=2,
                                  space="PSUM") as sgps, \
                     tc.tile_pool(name=f"aux{li}", bufs=3,
                                  space="PSUM") as auxps, \
                     tc.tile_pool(name=f"er{li}", bufs=3,
                                  space="PSUM") as erps:

                    def edge_pass(sched, idx_sb, smT_in, tab, passB,
                                  mkey, on_back_done=None, front_hook=None):
                        seg_tiles = {}
                        rs_left = [len(bl) for (bl, *_r) in RS_PLAN]
                        rs_fired_at = [None] * len(RS_PLAN)
                        state = {}

                        def emit_front(ci):
                            # gather + mask stream + er matmuls for chunk ci;
                            # then derive sm = transpose(smT) on the PE for
                            # the accumulate matmuls two chunks later.
                            (t0, nt, m0, nm) = sched["chunks"][ci]
                            g = gp.tile([128, CHUNK, TCOLS], bf16, tag="g")
                            nc.gpsimd.dma_gather(
                                out_ap=g[:, 0:nt, :], in_ap=tab[:, :],
                                idxs_ap=idx_sb[:, t0 * 8:(t0 + nt) * 8],
                                num_idxs=nt * 128, num_idxs_reg=nt * 128,
                                elem_size=TCOLS, single_packet=False)
                            smT = load_masks(mkey, sched, smT_in, ci)
                            er_ps = erps.tile([128, CHUNK, 4], f32,
                                              space="PSUM", tag="er_ps")
                            for t in range(nt):
                                for (ms, b, ef, el_, _sf, _sl) in \
                                        sched["tiles"][t0 + t]:
                                    lm = ms - m0
                                    nc.tensor.matmul(
                                        er_ps[:, t, :],
                                        lhsT=smT[:, lm * 128:(lm + 1) * 128],
                                        rhs=er_sb[li % 2][int(RNG_OF[b])][
                                            :, int(LROW_OF[b]), :],
                                        start=ef, stop=el_)
                            sm = mkp.tile([128, NM_MAX * 128], bf16, tag="sm")
                            for lm in range(nm):
                                st = auxps.tile([128, 136], f32,
                                                space="PSUM", tag="aux")
                                nc.tensor.matmul(
                                    st[:, 0:128],
                                    lhsT=smT[:, lm * 128:(lm + 1) * 128],
                                    rhs=identb_sb[:], start=True, stop=True)
                                nc.scalar.activation(
                                    sm[:, lm * 128:(lm + 1) * 128],
                                    st[:, 0:128], AF.Copy)
                            state[ci] = (t0, nt, m0, g, sm, er_ps)

                        def emit_back(ci):
                            (t0, nt, m0, g, sm, er_ps) = state.pop(ci)
                            for g0 in range(0, nt, GROUP):
                                gl = min(GROUP, nt - g0)
                                e4 = xp.tile([128, GROUP, 4], f32, tag="e4")
                                nc.vector.tensor_tensor(
                                    out=e4[:, 0:gl, :],
                                    in0=g[:, g0:g0 + gl, 128:132],
                                    in1=er_ps[:, g0:g0 + gl, :], op=OP.add)
                                lr = xp.tile([128, GROUP, 4], f32, tag="lr")
                                nc.scalar.activation(lr[:, 0:gl, :],
                                                     e4[:, 0:gl, :],
                                                     AF.Prelu, alpha=NEG)
                                ex4 = xp.tile([128, GROUP, 4, 1], f32,
                                              tag="ex4")
                                nc.scalar.activation(ex4[:, 0:gl, :, 0],
                                                     lr[:, 0:gl, :], AF.Exp)
                                m4 = mp.tile([128, GROUP, 132], bf16, tag="m4")
                                nc.scalar.activation(m4[:, 0:gl, 128:132],
                                                     ex4[:, 0:gl, :, 0],
                                                     AF.Copy)
                                nc.vector.tensor_tensor(
                                    out=m4[:, 0:gl, 0:128],
                                    in0=g[:, g0:g0 + gl, 0:128],
                                    in1=ex4[:, 0:gl, :, :]
                                        .to_broadcast([128, gl, 4, 32]),
                                    op=OP.mult)
                                for t in range(gl):
                                    for (ms, b, _ef, _el, sf, sl) in \
                                            sched["tiles"][t0 + g0 + t]:
                                        lm = ms - m0
                                        if sf:
                                            seg_tiles[b] = sgps.tile(
                                                [128, 132], f32, space="PSUM",
                                                tag="seg",
                                                name=f"seg{li}_{passB}_{b}")
                                        nc.tensor.matmul(
                                            seg_tiles[b][:],
                                            lhsT=sm[:, lm * 128:(lm + 1) * 128],
                                            rhs=m4[:, t, :],
                                            start=sf, stop=sl)
                                        if sl:
                                            ri_ = int(RNG_OF[b])
                                            lr = int(LROW_OF[b])
                                            if passB:
                                                pb = pbp.tile([128, 132], f32,
                                                              tag="pb")
                                                nc.vector.tensor_tensor(
                                                    out=pb[:],
                                                    in0=seg_tiles[b][:],
                                                    in1=paA[:, int(ROW_OF[b]),
                                                            :],
                                                    op=OP.add)
                                                nc.sync.dma_start(
                                                    out=partial[ri_][
                                                        lr * 128:
                                                        (lr + 1) * 128, :],
                                                    in_=pb[:])
                                                (bl, p_lo, p_hi, so_lo,
                                                 so_hi) = RS_PLAN[ri_]
                                                rs_left[ri_] -= 1
                                                if rs_left[ri_] == 0:
                                                    if not no_cc:
                                                        nc.gpsimd.\
                                                            collective_compute(
                                                            "ReduceScatter",
                                                            mybir.AluOpType.add,
                                                            replica_groups=
                                                            groups_pair,
                                                            ins=[partial[ri_]
                                                                 [:, :]],
                                                            outs=[own_sum[
                                                                so_lo:so_hi,
                                                                :]])
                                                    else:
                                                        sz = so_hi - so_lo
                                                        nc.sync.dma_start(
                                                            out=own_sum[
                                                                so_lo:so_hi,
                                                                :],
                                                            in_=partial[ri_][
                                                                0:sz, :])
                                                    rs_fired_at[ri_] = ci
                                            else:
                                                nc.scalar.activation(
                                                    paA[:, int(ROW_OF[b]), :],
                                                    seg_tiles[b][:], AF.Copy)
                                            seg_tiles.pop(b)
                            if on_back_done is not None:
                                on_back_done(ci, rs_fired_at)

                        nchunk = len(sched["chunks"])
                        AHEAD = 2
                        for ci in range(min(AHEAD, nchunk)):
                            emit_front(ci)
                            if front_hook is not None:
                                front_hook(ci)
                        for ci in range(AHEAD, nchunk):
                            emit_front(ci)
                            if front_hook is not None:
                                front_hook(ci)
                            emit_back(ci - AHEAD)
                        for ci in range(max(0, nchunk - AHEAD), nchunk):
                            emit_back(ci)

                    # post/proj streaming behind the split RS pieces.
                    # RS_DELAY chunks of slack before emitting each range's
                    # post ops so engine queues never stall on the RS sem.
                    RS_DELAY = [3, 2, 2, 0]
                    posted = [False] * len(RS_PLAN)
                    prefA_done = [False]

                    with tc.tile_pool(name=f"pr{li}", bufs=2) as prp:
                        prps = auxps

                        def on_back_done(ci, rs_fired_at):
                            nchunk = len(schedB["chunks"])
                            # prefetch next layer's pass-A masks mid-pass-B
                            if not last and not prefA_done[0] and \
                                    ci >= nchunk // 2:
                                prefetch_masks(f"A{li + 1}", schedA, smTA_in)
                                prefA_done[0] = True
                            for ri in range(len(RS_PLAN)):
                                if posted[ri] or rs_fired_at[ri] is None:
                                    continue
                                ready = (rs_fired_at[ri] + RS_DELAY[ri] <= ci
                                         or ci == nchunk - 1)
                                if ready and (ri == 0 or posted[ri - 1]):
                                    post_range(li, ri, prp, prps)
                                    posted[ri] = True

                        def front_hook_A(ci):
                            if li > 0 and ci == 5:
                                restripe_piece(li, 2)
                            elif li > 0 and ci == 7:
                                restripe_piece(li, 3)

                        edge_pass(schedA, idxA_sb, smTA_in, ag_own,
                                  False, f"A{li}", front_hook=front_hook_A)
                        prefetch_masks(f"B{li}", schedB, smTB_in)
                        edge_pass(schedB, idxB_sb, smTB_in,
                                  table[li % 2], True, f"B{li}", on_back_done)
                        for ri in range(len(RS_PLAN)):
                            if not posted[ri]:
                                post_range(li, ri, prp, prps)
                                posted[ri] = True
    nc.finalize()
    return nc


def _make_consts(W0, al0, ar0, b0, W1, al1, ar1, b1, W2, al2, ar2, b2):
    consts = {}
    for li, (W, al, ar, b) in enumerate(
            [(W0, al0, ar0, b0), (W1, al1, ar1, b1), (W2, al2, ar2, b2)]):
        consts[f"Waug{li}"] = _augment(np.asarray(W, np.float32),
                                       np.asarray(al, np.float32),
                                       np.asarray(ar, np.float32)).astype(
                                           ml_dtypes.bfloat16)
        b = np.asarray(b, np.float32)
        if li < 2:
            consts[f"brep{li}"] = np.tile(b.reshape(1, 128), (128, 1))
        else:
            consts[f"brep{li}"] = np.tile(b.reshape(H, D).mean(0).reshape(1, D),
                                          (128, 1))
    dummy = np.zeros((1, TCOLS), ml_dtypes.bfloat16)
    dummy[0, 128:132] = ml_dtypes.bfloat16(-1e30)
    consts["dummyrow"] = dummy
    return consts


def _in_maps(x):
    cores = _cache["pre"][0]
    x = np.asarray(x, dtype=np.float32)
    in_maps = []
    for c in range(NCORE):
        lo, hi = _own_rows(c)
        xT = np.zeros((128, OWN), ml_dtypes.bfloat16)
        xT[:, 0:NPC] = x[lo:hi].T.astype(ml_dtypes.bfloat16)
        cc = cores[c]
        in_maps.append(dict(xT=xT, idxA=cc["idxA"], idxB=cc["idxB"],
                            smTA=np.asarray(cc["smTA"]),
                            smTB=np.asarray(cc["smTB"])))
    return in_maps


def kernel(x, src, dst, W0, al0, ar0, b0, W1, al1, ar1, b1, W2, al2, ar2, b2):
    from concourse.bass_utils import run_bass_kernel_spmd

    key = (hash(np.asarray(src).tobytes()) ^ hash(np.asarray(dst).tobytes()))
    if "pre" not in _cache or _cache.get("prekey") != key:
        _cache["pre"] = _preprocess(src, dst)
        _cache["prekey"] = key
    cores, schedA, schedB = _cache["pre"]

    consts = _make_consts(W0, al0, ar0, b0, W1, al1, ar1, b1, W2, al2, ar2, b2)

    ck = key ^ hash(consts["Waug0"].tobytes())
    if "nc" not in _cache or _cache.get("nckey") != ck:
        _cache["nc"] = _build(schedA, schedB, consts)
        _cache["nckey"] = ck
    nc = _cache["nc"]

    in_maps = _in_maps(x)
    r = run_bass_kernel_spmd(nc, in_maps, list(range(NCORE)))
    y = np.zeros((N, D), np.float32)
    for c in range(NCORE):
        lo, hi = _own_rows(c)
        y[lo:hi] = r.results[c]["y"]
    return y


# revision 74
# speedup vs baseline: 1.1174x; 1.0056x over previous
"""3-layer GAT on 8 Trainium2 NeuronCores (Bass/Tile).

Sharding: 2D graph partition. Pair q = cores {2q, 2q+1} aggregates the dst
nodes of strips [q*6250,(q+1)*6250) and [25000+q*6250, 25000+(q+1)*6250);
even cores take edges with src < 25000, odd cores the rest. Node ownership:
core 2k owns rows [k*6250,(k+1)*6250), core 2k+1 owns [25000+k*6250, ...).

Per layer: each core projects its own rows (feat|el|er via an augmented
weight matrix) into a local gather table, then runs the edge phase in two
passes: pass A covers edges whose source is one of the core's own rows and
gathers from the local table while the quad AllGather of the full src-half
table is still in flight; pass B covers the remaining edges and gathers
from the AllGathered table. Edges are packed into 128-wide tiles grouped
by pairs of 128-dst blocks (a tile may straddle the two blocks; the
host-precomputed one-hot masks select membership). Per-edge er comes from
a transposed one-hot matmul against SBUF-resident per-block er rows (no
second gather). Messages are accumulated per dst block by one-hot-mask
matmuls into PSUM; pass B adds pass A's partial sums back in.

The dst pair-groups are processed in 4 segments; the pairwise
ReduceScatter of partial sums is split into 4 matching pieces (the
partial-row permutation interleaves [A-range | B-range] per piece so each
piece is contiguous), each fired as soon as its blocks complete mid-pass-B.
Post-processing (divide/bias/ELU, head-mean on the last layer) and the
next layer's projection are streamed per own-block range behind each RS
piece, so almost the whole layer boundary hides under pass B. The er
AllGather is split per range the same way (er_tab shares the partial-row
permutation, keeping each piece contiguous); mask loads for the first
chunks of each pass are prefetched during the previous pass. Gather calls
carry trailing -1 indices so the Q7 descriptor generator truncates padded
tail slots.
"""

import numpy as np
import ml_dtypes

N = 50000
E = 800000
F = 128                  # input feats and hidden width (4 heads x 32)
H = 4
D = 32
NEG = 0.2
NCORE = 8
NPC = 6250               # nodes owned per core
OWN = 6272               # 49*128, padded own rows
OWNBLK = 49
PAIR = 12544             # 98*128 dst slots per pair
NBLK = 98
NGRP = 49                # pair-groups of 2 blocks
HALF = 25088             # 4*OWN rows per src-half table
TROWS = 25216            # HALF + 128 (dummy row at HALF)
DUMMY = HALF
TCOLS = 256              # bf16 cols: feat(128) | el(4) | pad
PCOLS = 132              # packed AllGather row: feat(128) | el(4)
CHUNK = 16               # max tiles per dma_gather call
GROUP = 8                # tiles per vector-op batch
EPS = 1e-30

# Own-block ranges: post/proj are streamed per range, each behind its own
# ReduceScatter piece. The last range is small so the layer-boundary chain
# (last RS piece -> post -> proj -> table write) is short.
OWN_RANGES = [(0, 22), (22, 34), (34, 45), (45, 49)]

# pass-B pair-group order: 4 segments, segment r completes the blocks of
# OWN_RANGES[r] (A side: blocks lo..hi-1, B side: 49+lo..49+hi-1), so each
# ReduceScatter piece fires as early as possible. Straddle groups whose
# later-range block completes early are harmless (counters are per block).
GORDER_B = (list(range(0, 11)) + list(range(24, 36)) +    # seg 1
            list(range(11, 17)) + list(range(36, 42)) +   # seg 2
            list(range(17, 23)) + list(range(42, 47)) +   # seg 3
            [23, 47, 48])                                 # seg 4

# pass-A pair-group order: sorted by the er-AllGather piece each group
# needs (max over its two blocks), so groups needing the late pieces sit at
# the end of pass A and never stall the in-order engine queues.
GORDER_A = (list(range(0, 11)) + list(range(25, 35)) +    # er range 1
            list(range(11, 17)) + list(range(35, 41)) +   # er range 2
            list(range(17, 22)) + list(range(41, 47)) +   # er range 3
            [22, 23, 24, 47, 48])                         # er range 4

# block -> partial-row-block permutation: [A1 B1 | A2 B2 | A3 B3 | A4 B4]
# where Ar/Br are the A/B-side blocks of OWN_RANGES[r]. The same layout is
# used for er_tab so the split pair-AllGather outputs stay contiguous.
ROW_OF = np.empty(NBLK, np.int64)
RNG_OF = np.empty(NBLK, np.int64)    # block -> range index
LROW_OF = np.empty(NBLK, np.int64)   # block -> row-block within its range
RS_PLAN = []             # (blockset, p_lo, p_hi, o_lo, o_hi) per range
_base = 0
for _ri, (_lo, _hi) in enumerate(OWN_RANGES):
    _sz = _hi - _lo
    ROW_OF[_lo:_hi] = _base + np.arange(_sz)
    ROW_OF[49 + _lo:49 + _hi] = _base + _sz + np.arange(_sz)
    RNG_OF[_lo:_hi] = _ri
    RNG_OF[49 + _lo:49 + _hi] = _ri
    LROW_OF[_lo:_hi] = np.arange(_sz)
    LROW_OF[49 + _lo:49 + _hi] = _sz + np.arange(_sz)
    _blocks = frozenset(range(_lo, _hi)) | frozenset(range(49 + _lo, 49 + _hi))
    RS_PLAN.append((_blocks, _base * 128, (_base + 2 * _sz) * 128,
                    _lo * 128, _hi * 128))
    _base += 2 * _sz
assert _base == NBLK

# Range-major layout of the quad-AllGathered src-half table: range r holds
# its 4 ranks' own-row stripes contiguously, so the table AllGather (and
# the local 264B->512B re-stripe) splits into 4 contiguous pieces.
T_PBASE = []
_tb = 0
for _lo, _hi in OWN_RANGES:
    T_PBASE.append(_tb)
    _tb += 4 * (_hi - _lo) * 128
assert _tb == HALF
# own-local row -> range-major table row offset (within one rank's stripe
# the rows keep own-local order; rank k of range r starts at
# T_PBASE[r] + k * sz_r * 128).
_OROW_RANGE = np.empty(OWN, np.int64)   # own row -> range idx
_OROW_LOCAL = np.empty(OWN, np.int64)   # own row -> row within range stripe
for _ri, (_lo, _hi) in enumerate(OWN_RANGES):
    _OROW_RANGE[_lo * 128:_hi * 128] = _ri
    _OROW_LOCAL[_lo * 128:_hi * 128] = np.arange((_hi - _lo) * 128)

_cache = {}


def _schedule(cnt, gorder):
    """Core-uniform tile/mask schedule for one pass.

    cnt: [NCORE, NBLK] per-core per-block edge counts.
    Returns dict with T, nmask, chunks, tiles (per tile: list of
    (mslot, block, er_first, er_last, sc_first, sc_last)).
    """
    GORDER = gorder
    n0 = cnt[:, 0::2]                      # [NCORE, NGRP]
    n1 = cnt[:, 1::2]
    TP = np.maximum(1, np.ceil((n0 + n1).max(axis=0) / 128).astype(np.int64))

    base_tile = {}
    acc = 0
    for g in GORDER:
        base_tile[g] = acc
        acc += int(TP[g])
    T = acc

    tiles = []            # per tile: list of [mslot, block]
    tile_group = []
    # which (tile-in-group, block-parity) pairs are needed on any core; ensure
    # every block gets at least one occurrence (tile 0 fallback)
    need = {}
    for g in GORDER:
        for i in range(int(TP[g])):
            need[(g, i, 0)] = bool((n0[:, g] > 128 * i).any())
            need[(g, i, 1)] = bool(
                ((n0[:, g] < 128 * (i + 1)) &
                 (n0[:, g] + n1[:, g] > 128 * i)).any())
        if not any(need[(g, i, 0)] for i in range(int(TP[g]))):
            need[(g, 0, 0)] = True
        if not any(need[(g, i, 1)] for i in range(int(TP[g]))):
            need[(g, 0, 1)] = True
    mslot = 0
    for g in GORDER:
        for i in range(int(TP[g])):
            ml = []
            if need[(g, i, 0)]:
                ml.append([mslot, 2 * g])
                mslot += 1
            if need[(g, i, 1)]:
                ml.append([mslot, 2 * g + 1])
                mslot += 1
            assert ml
            tiles.append(ml)
            tile_group.append(g)
    nmask = mslot

    # per-block first/last occurrence
    occ = {}
    for ti, ml in enumerate(tiles):
        for m in ml:
            occ.setdefault(m[1], []).append((ti, m[0]))
    first = {b: o[0] for b, o in occ.items()}
    last = {b: o[-1] for b, o in occ.items()}
    sched_tiles = []
    for ti, ml in enumerate(tiles):
        entry = []
        for k, (ms, b) in enumerate(ml):
            entry.append((ms, b,
                          k == 0, k == len(ml) - 1,
                          first[b] == (ti, ms), last[b] == (ti, ms)))
        sched_tiles.append(entry)

    # chunks aligned to pair-group boundaries, up to CHUNK tiles
    chunks = []
    t0 = 0
    ti = 0
    for g in GORDER:
        ti += int(TP[g])
        nxt = None
        gi = GORDER.index(g)
        if gi + 1 < len(GORDER):
            nxt = int(TP[GORDER[gi + 1]])
        if nxt is None or ti - t0 + nxt > CHUNK:
            m0 = min(m[0] for m in sched_tiles[t0]) if sched_tiles[t0] else 0
            mend = max(m[0] for m in sched_tiles[ti - 1]) + 1
            chunks.append((t0, ti - t0, m0, mend - m0))
            t0 = ti
    assert t0 == T
    return dict(T=T, nmask=nmask, chunks=chunks, tiles=sched_tiles,
                tile_group=tile_group, base_tile=base_tile, TP=TP,
                gorder=gorder)


def _wrap16(a):
    # value i of each 128-group at [i%16, i//16], replicated per 16 rows
    t = a.reshape(-1, 128)                     # [T, 128]
    w = t.reshape(t.shape[0], 8, 16)           # [T, 8, 16]
    w = w.transpose(2, 0, 1).reshape(16, -1)   # [16, T*8]
    return np.tile(w, (8, 1)).astype(np.int16)  # [128, T*8]


def _core_pass_arrays(sched, rloc_e, rows_e, pad_row, trunc_from_chunk):
    """Build idx + mask streams for one (core, pass).

    rloc_e: pair-local dst row per edge; rows_e: gather-table row per edge.
    Chunks with index >= trunc_from_chunk get their trailing padded idx
    slots set to -1 (the Q7 truncates them); earlier chunks keep the dummy
    row so first-touch SBUF tiles never expose uninitialized data.
    """
    T, nmask = sched["T"], sched["nmask"]
    base_tile = sched["base_tile"]
    GORDER = sched["gorder"]
    # group rank of each edge
    grank_of = np.empty(NGRP, np.int64)
    for r, g in enumerate(GORDER):
        grank_of[g] = r
    pg = rloc_e // 256
    gr = grank_of[pg]
    order = np.lexsort((rloc_e, gr))
    rloc_s = rloc_e[order]
    rows_s = rows_e[order]
    gr_s = gr[order]
    # position within group
    starts = np.searchsorted(gr_s, np.arange(len(GORDER)))
    pos_in_group = np.arange(len(gr_s)) - starts[gr_s]
    base128 = np.array([base_tile[GORDER[r]] * 128
                        for r in range(len(GORDER))], np.int64)
    s_glob = base128[gr_s] + pos_in_group

    idx = np.full(T * 128, pad_row, np.int64)
    idx[s_glob] = rows_s
    real = np.zeros(T * 128, bool)
    real[s_glob] = True
    for ci, (t0, nt, _m0, _nm) in enumerate(sched["chunks"]):
        if ci < trunc_from_chunk:
            continue
        a, b = t0 * 128, (t0 + nt) * 128
        nz = np.flatnonzero(real[a:b])
        last = nz[-1] if len(nz) else -1
        idx[a + last + 1:b] = -1

    # mask slot lookup per (tile, block-parity)
    mslot_of = np.full((T, 2), -1, np.int64)
    for ti, ml in enumerate(sched["tiles"]):
        g = sched["tile_group"][ti]
        for (ms, b, *_fl) in ml:
            mslot_of[ti, b - 2 * g] = ms
    ti_e = s_glob // 128
    e_e = s_glob % 128
    b_e = rloc_s // 128
    s128_e = rloc_s % 128
    par = b_e - 2 * np.array(sched["tile_group"])[ti_e]
    ms_e = mslot_of[ti_e, par]
    assert (ms_e >= 0).all()

    smatTw = np.zeros((128, nmask * 128), ml_dtypes.bfloat16)
    smatTw[s128_e, ms_e * 128 + e_e] = 1
    return _wrap16(idx), smatTw


def _preprocess(src, dst):
    src = np.asarray(src).astype(np.int64)
    dst = np.asarray(dst).astype(np.int64)
    q = np.where(dst < 25000, dst // NPC, (dst - 25000) // NPC)
    s = (src >= 25000).astype(np.int64)
    core_of = 2 * q + s
    rloc = np.where(dst < 25000, dst - q * NPC, OWN + (dst - 25000 - q * NPC))
    ks = np.where(src < 25000, src // NPC, (src - 25000) // NPC)
    olocal = np.where(src < 25000, src - ks * NPC, (src - 25000) - ks * NPC)
    _ri = _OROW_RANGE[olocal]
    _szr = np.array([(hi - lo) * 128 for lo, hi in OWN_RANGES], np.int64)
    tloc = (np.array(T_PBASE, np.int64)[_ri] + ks * _szr[_ri]
            + _OROW_LOCAL[olocal])
    own = ks == q
    ownrow = np.where(src < 25000, src - ks * NPC, src - 25000 - ks * NPC)
    blk = rloc // 128

    cntA = np.zeros((NCORE, NBLK), np.int64)
    cntB = np.zeros((NCORE, NBLK), np.int64)
    for c in range(NCORE):
        m = core_of == c
        cntA[c] = np.bincount(blk[m & own], minlength=NBLK)
        cntB[c] = np.bincount(blk[m & ~own], minlength=NBLK)
    schedA = _schedule(cntA, GORDER_A)
    schedB = _schedule(cntB, GORDER_B)

    cores = []
    for c in range(NCORE):
        m = core_of == c
        mA = m & own
        mB = m & ~own
        idxA, smTA = _core_pass_arrays(schedA, rloc[mA], ownrow[mA],
                                       OWN, 10**9)
        idxB, smTB = _core_pass_arrays(schedB, rloc[mB], tloc[mB],
                                       DUMMY, 10**9)
        cores.append(dict(idxA=idxA, smTA=smTA, idxB=idxB, smTB=smTB))
    return cores, schedA, schedB


def _own_rows(c):
    k = c // 2
    if c % 2 == 0:
        return k * NPC, (k + 1) * NPC
    return 25000 + k * NPC, 25000 + (k + 1) * NPC


def _augment(W, al, ar):
    dout = W.shape[1] // H
    Wal = np.stack([W[:, h * dout:(h + 1) * dout] @ al[h] for h in range(H)], 1)
    War = np.stack([W[:, h * dout:(h + 1) * dout] @ ar[h] for h in range(H)], 1)
    return np.concatenate([W, Wal, War], 1).astype(np.float32)  # [128, 136]


def _build(schedA, schedB, consts, no_cc=False):
    import concourse.bass as bass
    import concourse.bacc as bacc
    import concourse.tile as tile
    from concourse import mybir
    from concourse.library_config import mlp

    f32 = mybir.dt.float32
    bf16 = mybir.dt.bfloat16
    i16 = mybir.dt.int16
    AF = mybir.ActivationFunctionType
    OP = mybir.AluOpType

    TA, TB = schedA["T"], schedB["T"]
    NMA, NMB = schedA["nmask"], schedB["nmask"]
    NM_MAX = max(max(nm for (_, _, _, nm) in schedA["chunks"]),
                 max(nm for (_, _, _, nm) in schedB["chunks"]))

    nc = bacc.Bacc(num_devices=NCORE)
    xT_in = nc.declare_dram_parameter("xT", [128, OWN], bf16, isOutput=False)
    idxA_in = nc.declare_dram_parameter("idxA", [128, TA * 8], i16,
                                        isOutput=False)
    idxB_in = nc.declare_dram_parameter("idxB", [128, TB * 8], i16,
                                        isOutput=False)
    smTA_in = nc.declare_dram_parameter("smTA", [128, NMA * 128], bf16,
                                        isOutput=False)
    smTB_in = nc.declare_dram_parameter("smTB", [128, NMB * 128], bf16,
                                        isOutput=False)
    y_out = nc.declare_dram_parameter("y", [NPC, D], f32, isOutput=True)

    with tile.TileContext(nc) as tc:
        with tc.tile_pool(name="persist", bufs=1) as pp, \
             tc.tile_pool(name="mk", bufs=3) as mkp, \
             tc.tile_pool(name="dram", bufs=1, space="DRAM") as dp:
            nc.gpsimd.load_library(mlp)

            # ---- persistent SBUF state ----
            idxA_sb = pp.tile([128, TA * 8], i16)
            nc.sync.dma_start(out=idxA_sb[:], in_=idxA_in[:, :])
            idxB_sb = pp.tile([128, TB * 8], i16)
            nc.sync.dma_start(out=idxB_sb[:], in_=idxB_in[:, :])
            hT = pp.tile([128, OWN], bf16)
            hT2 = pp.tile([128, OWN], bf16)

            ident_h = nc.inline_tensor(np.eye(128, dtype=np.float32),
                                       name="ident")
            ident_sb = pp.tile([128, 128], f32)
            nc.sync.dma_start(out=ident_sb[:], in_=ident_h[:, :])
            identb_h = nc.inline_tensor(np.eye(128).astype(ml_dtypes.bfloat16),
                                        name="identb")
            identb_sb = pp.tile([128, 128], bf16)
            nc.sync.dma_start(out=identb_sb[:], in_=identb_h[:, :])

            waug_sb = []
            brep_sb = []
            for li in range(3):
                wh = nc.inline_tensor(consts[f"Waug{li}"], name=f"waug{li}")
                wt = pp.tile([128, 136], bf16, name=f"waug_sb{li}")
                nc.sync.dma_start(out=wt[:], in_=wh[:, :])
                waug_sb.append(wt)
                bh = nc.inline_tensor(consts[f"brep{li}"], name=f"brep{li}")
                bt = pp.tile([128, consts[f"brep{li}"].shape[1]], f32,
                             name=f"brep_sb{li}")
                nc.sync.dma_start(out=bt[:], in_=bh[:, :])
                brep_sb.append(bt)

            dummy_h = nc.inline_tensor(consts["dummyrow"], name="dummyrow")

            # ---- DRAM scratch ----
            # partialA/partial are split per own-block range so pass-B loads
            # of a range only depend on that range's pass-A writes (and the
            # RS pieces only on their own range's writes).
            # The gather table is double buffered by layer parity (the next
            # layer's split AllGather pieces land while the current layer's
            # pass B still gathers from the old table). The AllGather moves
            # packed 264B rows; a local re-stripe expands to the 512B-stride
            # layout the gather needs.
            table = [dp.tile([TROWS, TCOLS], bf16, name=f"table{p}")
                     for p in range(2)]
            table_pack = dp.tile([HALF, PCOLS], bf16)
            er_tab = dp.tile([PAIR, 4], bf16)
            ag_own = dp.tile([OWN + 128, TCOLS], bf16)
            ag_pack = dp.tile([OWN, PCOLS], bf16)
            ag_er = dp.tile([OWN, 4], bf16)
            partial = [dp.tile([(p_hi - p_lo), 132], f32, name=f"partial{ri}")
                       for ri, (_b, p_lo, p_hi, _o, _o2) in
                       enumerate(RS_PLAN)]
            own_sum = dp.tile([OWN, 132], f32)

            nc.sync.dma_start(out=table[0][DUMMY:DUMMY + 1, :],
                              in_=dummy_h[:, :])
            nc.sync.dma_start(out=table[1][DUMMY:DUMMY + 1, :],
                              in_=dummy_h[:, :])
            nc.sync.dma_start(out=ag_own[OWN:OWN + 1, :], in_=dummy_h[:, :])

            groups_pair = [[2 * k, 2 * k + 1] for k in range(4)]
            groups_quad = [[0, 2, 4, 6], [1, 3, 5, 7]]

            # mask prefetch bookkeeping: (pass_key, ci) -> smT tile. Only
            # smT lives in DRAM; sm is derived on-chip by PE transpose in
            # the front->back slack of the chunk pipeline.
            premask = {}

            def load_masks(key, sched, smT_in, ci):
                if (key, ci) in premask:
                    return premask.pop((key, ci))
                (_t0, _nt, m0, nm) = sched["chunks"][ci]
                smT = mkp.tile([128, NM_MAX * 128], bf16, tag="smT")
                nc.sync.dma_start(
                    out=smT[:, 0:nm * 128],
                    in_=smT_in[:, m0 * 128:(m0 + nm) * 128])
                return smT

            def prefetch_masks(key, sched, smT_in, n=2):
                for ci in range(min(n, len(sched["chunks"]))):
                    premask[(key, ci)] = load_masks(key + "_", sched,
                                                    smT_in, ci)

            # pass-A partial sums stay SBUF-resident (bf16): pass B adds
            # them back without any DRAM round-trip.
            paA = pp.tile([128, NBLK, 132], bf16, name="paA")

            # er_sb: per-layer er rows in partial-permuted block order, one
            # tile per range so readers only depend on their own range's
            # split er AllGather. Double buffered by layer parity: the next
            # layer's er loads land while the current layer's edge passes
            # still read the old values.
            er_sb = [[pp.tile([128, (p_hi - p_lo) // 128, 4], bf16,
                              name=f"er_sb{par}_{ri}")
                      for ri, (_b, p_lo, p_hi, _o, _o2) in
                      enumerate(RS_PLAN)]
                     for par in range(2)]

            def load_er_range(li, r):
                (_bl, p_lo, p_hi, _o_lo, _o_hi) = RS_PLAN[r]
                nc.sync.dma_start(
                    out=er_sb[li % 2][r][:],
                    in_=er_tab[p_lo:p_hi, :]
                        .rearrange("(t p) c -> p t c", p=128))

            def proj_range(li, src_hT, r, prange_pool, prj_ps):
                """Project own blocks of OWN_RANGES[r]: write ag_own rows,
                ag_er rows, then fire the split pair-AllGather of er."""
                (o_lo, o_hi) = OWN_RANGES[r]
                for b0 in range(o_lo, o_hi, 4):
                    nb = min(8, o_hi - b0)
                    tabrow = prange_pool.tile([128, 4, TCOLS], bf16,
                                              tag="tabrow")
                    errow = prange_pool.tile([128, 4, 4], bf16, tag="errow")
                    for t in range(nb):
                        pj = prj_ps.tile([128, 136], f32, space="PSUM",
                                         tag="aux")
                        nc.tensor.matmul(
                            pj[:],
                            lhsT=src_hT[:, (b0 + t) * 128:(b0 + t + 1) * 128],
                            rhs=waug_sb[li][:, 0:136], start=True, stop=True)
                        nc.scalar.activation(tabrow[:, t, 0:132],
                                             pj[:, 0:132], AF.Copy)
                        nc.scalar.activation(errow[:, t, :], pj[:, 132:136],
                                             AF.Copy)
                    nc.sync.dma_start(
                        out=ag_own[b0 * 128:(b0 + nb) * 128, :]
                            .rearrange("(t p) c -> p t c", p=128),
                        in_=tabrow[:, 0:nb, :])
                    nc.sync.dma_start(
                        out=ag_pack[b0 * 128:(b0 + nb) * 128, :]
                            .rearrange("(t p) c -> p t c", p=128),
                        in_=tabrow[:, 0:nb, 0:PCOLS])
                    nc.sync.dma_start(
                        out=ag_er[b0 * 128:(b0 + nb) * 128, :]
                            .rearrange("(t p) c -> p t c", p=128),
                        in_=errow[:, 0:nb, :])
                (_bl, p_lo, p_hi, ro_lo, ro_hi) = RS_PLAN[r]
                if no_cc:
                    sz = ro_hi - ro_lo
                    nc.sync.dma_start(out=er_tab[p_lo:p_lo + sz, :],
                                      in_=ag_er[ro_lo:ro_hi, :])
                    nc.sync.dma_start(out=er_tab[p_lo + sz:p_hi, :],
                                      in_=ag_er[ro_lo:ro_hi, :])
                else:
                    nc.gpsimd.collective_compute(
                        "AllGather", mybir.AluOpType.bypass,
                        replica_groups=groups_pair,
                        ins=[ag_er[ro_lo:ro_hi, :]],
                        outs=[er_tab[p_lo:p_hi, :]])
                load_er_range(li, r)

            def table_ag_piece(li, r):
                """Quad-AllGather the packed own rows of range r into the
                contiguous packed table."""
                (o_lo, o_hi) = OWN_RANGES[r]
                sz = (o_hi - o_lo) * 128
                t0 = T_PBASE[r]
                if no_cc:
                    for rep in range(4):
                        nc.sync.dma_start(
                            out=table_pack[t0 + rep * sz:t0 + (rep + 1) * sz,
                                           :],
                            in_=ag_pack[o_lo * 128:o_hi * 128, :])
                else:
                    nc.gpsimd.collective_compute(
                        "AllGather", mybir.AluOpType.bypass,
                        replica_groups=groups_quad,
                        ins=[ag_pack[o_lo * 128:o_hi * 128, :]],
                        outs=[table_pack[t0:t0 + 4 * sz, :]])

            def restripe_piece(li, r):
                # 264B-packed -> 512B-stride expansion; emit only at points
                # where the matching AG piece is already complete, else the
                # in-order sync DMA queue bubbles behind the wait.
                (o_lo, o_hi) = OWN_RANGES[r]
                sz = (o_hi - o_lo) * 128
                t0 = T_PBASE[r]
                nc.sync.dma_start(
                    out=table[li % 2][t0:t0 + 4 * sz, 0:PCOLS],
                    in_=table_pack[t0:t0 + 4 * sz, :])

            def post_range(li, r, prange_pool, prj_ps):
                """Divide/bias/activation for own blocks of OWN_RANGES[r];
                for layers 0/1 follow with the next layer's projection of
                the same rows, for the last layer write y output rows."""
                last = li == 2
                (o_lo, o_hi) = OWN_RANGES[r]
                dst_hT = hT2 if li % 2 == 0 else hT
                for b0 in range(o_lo, o_hi, 4):
                    nb = min(8, o_hi - b0)
                    osum = prange_pool.tile([128, 4, 132], f32, tag="osum")
                    nc.sync.dma_start(
                        out=osum[:, 0:nb, :],
                        in_=own_sum[b0 * 128:(b0 + nb) * 128, :]
                            .rearrange("(t p) c -> p t c", p=128))
                    den = prange_pool.tile([128, 4, 4], f32, tag="den")
                    nc.vector.tensor_scalar_max(den[:, 0:nb, :],
                                                osum[:, 0:nb, 128:132], EPS)
                    rec = prange_pool.tile([128, 4, 4, 1], f32, tag="rec")
                    nc.vector.reciprocal(rec[:, 0:nb, :, 0], den[:, 0:nb, :])
                    if not last:
                        o2 = prange_pool.tile([128, 4, 128], f32, tag="o2")
                        nc.vector.tensor_tensor(
                            out=o2[:, 0:nb, :]
                                .rearrange("p t (h d) -> p t h d", h=4),
                            in0=osum[:, 0:nb, 0:128]
                                .rearrange("p t (h d) -> p t h d", h=4),
                            in1=rec[:, 0:nb, :, :]
                                .to_broadcast([128, nb, 4, 32]),
                            op=OP.mult)
                        nc.vector.tensor_tensor(
                            out=o2[:, 0:nb, :], in0=o2[:, 0:nb, :],
                            in1=brep_sb[li][:]
                                .rearrange("p (t c) -> p t c", t=1)
                                .to_broadcast([128, nb, 128]),
                            op=OP.add)
                        # ELU via scalar engine: exn = exp(-relu(-x)) =
                        # exp(min(x,0)); o2 = relu(x) + exn - 1
                        exn = prange_pool.tile([128, 4, 128], f32, tag="exn")
                        nc.scalar.activation(exn[:, 0:nb, :], o2[:, 0:nb, :],
                                             AF.Relu, scale=-1.0)
                        nc.scalar.activation(exn[:, 0:nb, :], exn[:, 0:nb, :],
                                             AF.Exp, scale=-1.0)
                        nc.scalar.activation(o2[:, 0:nb, :], o2[:, 0:nb, :],
                                             AF.Relu)
                        nc.vector.tensor_tensor(out=o2[:, 0:nb, :],
                                                in0=o2[:, 0:nb, :],
                                                in1=exn[:, 0:nb, :],
                                                op=OP.add)
                        nc.scalar.activation(o2[:, 0:nb, :], o2[:, 0:nb, :],
                                             AF.Copy, bias=-1.0)
                        for t in range(nb):
                            tp = prj_ps.tile([128, 136], f32, space="PSUM",
                                             tag="aux")
                            nc.tensor.matmul(tp[:, 0:128], lhsT=o2[:, t, :],
                                             rhs=ident_sb[:], start=True,
                                             stop=True)
                            nc.scalar.activation(
                                dst_hT[:, (b0 + t) * 128:(b0 + t + 1) * 128],
                                tp[:, 0:128], AF.Copy)
                    else:
                        r4 = prange_pool.tile([128, 4, 4, 32], f32, tag="r4")
                        nc.vector.tensor_tensor(
                            out=r4[:, 0:nb, :, :],
                            in0=osum[:, 0:nb, 0:128]
                                .rearrange("p t (h d) -> p t h d", h=4),
                            in1=rec[:, 0:nb, :, :]
                                .to_broadcast([128, nb, 4, 32]),
                            op=OP.mult)
                        r1 = prange_pool.tile([128, 4, 32], f32, tag="r1")
                        nc.vector.tensor_tensor(out=r1[:, 0:nb, :],
                                                in0=r4[:, 0:nb, 0, :],
                                                in1=r4[:, 0:nb, 1, :],
                                                op=OP.add)
                        r2 = prange_pool.tile([128, 4, 32], f32, tag="r2")
                        nc.vector.tensor_tensor(out=r2[:, 0:nb, :],
                                                in0=r4[:, 0:nb, 2, :],
                                                in1=r4[:, 0:nb, 3, :],
                                                op=OP.add)
                        nc.vector.tensor_tensor(out=r1[:, 0:nb, :],
                                                in0=r1[:, 0:nb, :],
                                                in1=r2[:, 0:nb, :], op=OP.add)
                        nc.vector.tensor_scalar_mul(r1[:, 0:nb, :],
                                                    r1[:, 0:nb, :], 0.25)
                        nc.vector.tensor_tensor(
                            out=r1[:, 0:nb, :], in0=r1[:, 0:nb, :],
                            in1=brep_sb[li][:]
                                .rearrange("p (t c) -> p t c", t=1)
                                .to_broadcast([128, nb, 32]),
                            op=OP.add)
                        nfull = nb if (b0 + nb) * 128 <= NPC else nb - 1
                        if nfull > 0:
                            nc.sync.dma_start(
                                out=y_out[b0 * 128:(b0 + nfull) * 128, :]
                                    .rearrange("(t p) c -> p t c", p=128),
                                in_=r1[:, 0:nfull, :])
                        if nfull < nb:
                            rem = NPC - (b0 + nfull) * 128
                            nc.sync.dma_start(
                                out=y_out[(b0 + nfull) * 128:NPC, :],
                                in_=r1[0:rem, nfull, :])
                if not last:
                    proj_range(li + 1, dst_hT, r, prange_pool, prj_ps)
                    # table AG pieces: r=0,1 fire mid-pass-B (CC has slack
                    # there); r=2,3 are deferred past the RS pieces so they
                    # never delay the boundary-critical ReduceScatters.
                    if r <= 1:
                        table_ag_piece(li + 1, r)
                    elif r == 3:
                        table_ag_piece(li + 1, 2)
                        table_ag_piece(li + 1, 3)
                        # AG pieces 0/1 fired mid-pass-B and are done;
                        # restripe them now. Pieces 2/3 restripe during the
                        # next layer's pass A (front_hook).
                        restripe_piece(li + 1, 0)
                        restripe_piece(li + 1, 1)

            # ---- layer 0 init: load xT + streamed projection ----
            prefetch_masks("A0", schedA, smTA_in)
            with tc.tile_pool(name="prj0", bufs=2) as p0p, \
                 tc.tile_pool(name="prj0ps", bufs=3, space="PSUM") as p0ps:
                for r, (o_lo, o_hi) in enumerate(OWN_RANGES):
                    nc.sync.dma_start(
                        out=hT[:, o_lo * 128:o_hi * 128],
                        in_=xT_in[:, o_lo * 128:o_hi * 128])
                    proj_range(0, hT, r, p0p, p0ps)
                    table_ag_piece(0, r)
                for r in range(len(OWN_RANGES)):
                    restripe_piece(0, r)

            for li in range(3):
                last = li == 2

                with tc.tile_pool(name=f"gt{li}", bufs=8) as gp, \
                     tc.tile_pool(name=f"ms{li}", bufs=4) as mp, \
                     tc.tile_pool(name=f"ex{li}", bufs=4) as xp, \
                     tc.tile_pool(name=f"pb{li}", bufs=8) as pbp, \
                     tc.tile_pool(name=f"sg{li}", bufs=2,
                                  space="PSUM") as sgps, \
                     tc.tile_pool(name=f"aux{li}", bufs=3,
                                  space="PSUM") as auxps, \
                     tc.tile_pool(name=f"er{li}", bufs=3,
                                  space="PSUM") as erps:

                    def edge_pass(sched, idx_sb, smT_in, tab, passB,
                                  mkey, on_back_done=None, front_hook=None):
                        seg_tiles = {}
                        rs_left = [len(bl) for (bl, *_r) in RS_PLAN]
                        rs_fired_at = [None] * len(RS_PLAN)
                        state = {}

                        def emit_front(ci):
                            # gather + mask stream + er matmuls for chunk ci;
                            # then derive sm = transpose(smT) on the PE for
                            # the accumulate matmuls two chunks later.
                            (t0, nt, m0, nm) = sched["chunks"][ci]
                            g = gp.tile([128, CHUNK, TCOLS], bf16, tag="g")
                            nc.gpsimd.dma_gather(
                                out_ap=g[:, 0:nt, :], in_ap=tab[:, :],
                                idxs_ap=idx_sb[:, t0 * 8:(t0 + nt) * 8],
                                num_idxs=nt * 128, num_idxs_reg=nt * 128,
                                elem_size=TCOLS, single_packet=False)
                            smT = load_masks(mkey, sched, smT_in, ci)
                            er_ps = erps.tile([128, CHUNK, 4], f32,
                                              space="PSUM", tag="er_ps")
                            for t in range(nt):
                                for (ms, b, ef, el_, _sf, _sl) in \
                                        sched["tiles"][t0 + t]:
                                    lm = ms - m0
                                    nc.tensor.matmul(
                                        er_ps[:, t, :],
                                        lhsT=smT[:, lm * 128:(lm + 1) * 128],
                                        rhs=er_sb[li % 2][int(RNG_OF[b])][
                                            :, int(LROW_OF[b]), :],
                                        start=ef, stop=el_)
                            sm = mkp.tile([128, NM_MAX * 128], bf16, tag="sm")
                            for lm in range(nm):
                                st = auxps.tile([128, 136], f32,
                                                space="PSUM", tag="aux")
                                nc.tensor.matmul(
                                    st[:, 0:128],
                                    lhsT=smT[:, lm * 128:(lm + 1) * 128],
                                    rhs=identb_sb[:], start=True, stop=True)
                                nc.scalar.activation(
                                    sm[:, lm * 128:(lm + 1) * 128],
                                    st[:, 0:128], AF.Copy)
                            state[ci] = (t0, nt, m0, g, sm, er_ps)

                        def emit_back(ci):
                            (t0, nt, m0, g, sm, er_ps) = state.pop(ci)
                            for g0 in range(0, nt, GROUP):
                                gl = min(GROUP, nt - g0)
                                e4 = xp.tile([128, GROUP, 4], f32, tag="e4")
                                nc.vector.tensor_tensor(
                                    out=e4[:, 0:gl, :],
                                    in0=g[:, g0:g0 + gl, 128:132],
                                    in1=er_ps[:, g0:g0 + gl, :], op=OP.add)
                                lr = xp.tile([128, GROUP, 4], f32, tag="lr")
                                nc.scalar.activation(lr[:, 0:gl, :],
                                                     e4[:, 0:gl, :],
                                                     AF.Prelu, alpha=NEG)
                                ex4 = xp.tile([128, GROUP, 4, 1], f32,
                                              tag="ex4")
                                nc.scalar.activation(ex4[:, 0:gl, :, 0],
                                                     lr[:, 0:gl, :], AF.Exp)
                                m4 = mp.tile([128, GROUP, 132], bf16, tag="m4")
                                nc.scalar.activation(m4[:, 0:gl, 128:132],
                                                     ex4[:, 0:gl, :, 0],
                                                     AF.Copy)
                                nc.vector.tensor_tensor(
                                    out=m4[:, 0:gl, 0:128],
                                    in0=g[:, g0:g0 + gl, 0:128],
                                    in1=ex4[:, 0:gl, :, :]
                                        .to_broadcast([128, gl, 4, 32]),
                                    op=OP.mult)
                                for t in range(gl):
                                    for (ms, b, _ef, _el, sf, sl) in \
                                            sched["tiles"][t0 + g0 + t]:
                                        lm = ms - m0
                                        if sf:
                                            seg_tiles[b] = sgps.tile(
                                                [128, 132], f32, space="PSUM",
                                                tag="seg",
                                                name=f"seg{li}_{passB}_{b}")
                                        nc.tensor.matmul(
                                            seg_tiles[b][:],
                                            lhsT=sm[:, lm * 128:(lm + 1) * 128],
                                            rhs=m4[:, t, :],
                                            start=sf, stop=sl)
                                        if sl:
                                            ri_ = int(RNG_OF[b])
                                            lr = int(LROW_OF[b])
                                            if passB:
                                                pb = pbp.tile([128, 132], f32,
                                                              tag="pb")
                                                nc.vector.tensor_tensor(
                                                    out=pb[:],
                                                    in0=seg_tiles[b][:],
                                                    in1=paA[:, int(ROW_OF[b]),
                                                            :],
                                                    op=OP.add)
                                                nc.sync.dma_start(
                                                    out=partial[ri_][
                                                        lr * 128:
                                                        (lr + 1) * 128, :],
                                                    in_=pb[:])
                                                (bl, p_lo, p_hi, so_lo,
                                                 so_hi) = RS_PLAN[ri_]
                                                rs_left[ri_] -= 1
                                                if rs_left[ri_] == 0:
                                                    if not no_cc:
                                                        nc.gpsimd.\
                                                            collective_compute(
                                                            "ReduceScatter",
                                                            mybir.AluOpType.add,
                                                            replica_groups=
                                                            groups_pair,
                                                            ins=[partial[ri_]
                                                                 [:, :]],
                                                            outs=[own_sum[
                                                                so_lo:so_hi,
                                                                :]])
                                                    else:
                                                        sz = so_hi - so_lo
                                                        nc.sync.dma_start(
                                                            out=own_sum[
                                                                so_lo:so_hi,
                                                                :],
                                                            in_=partial[ri_][
                                                                0:sz, :])
                                                    rs_fired_at[ri_] = ci
                                            else:
                                                nc.scalar.activation(
                                                    paA[:, int(ROW_OF[b]), :],
                                                    seg_tiles[b][:], AF.Copy)
                                            seg_tiles.pop(b)
                            if on_back_done is not None:
                                on_back_done(ci, rs_fired_at)

                        nchunk = len(sched["chunks"])
                        AHEAD = 2
                        for ci in range(min(AHEAD, nchunk)):
                            emit_front(ci)
                            if front_hook is not None:
                                front_hook(ci)
                        for ci in range(AHEAD, nchunk):
                            emit_front(ci)
                            if front_hook is not None:
                                front_hook(ci)
                            emit_back(ci - AHEAD)
                        for ci in range(max(0, nchunk - AHEAD), nchunk):
                            emit_back(ci)

                    # post/proj streaming behind the split RS pieces.
                    # RS_DELAY chunks of slack before emitting each range's
                    # post ops so engine queues never stall on the RS sem.
                    RS_DELAY = [3, 2, 2, 0]
                    posted = [False] * len(RS_PLAN)
                    prefA_done = [False]

                    with tc.tile_pool(name=f"pr{li}", bufs=3) as prp:
                        prps = auxps

                        def on_back_done(ci, rs_fired_at):
                            nchunk = len(schedB["chunks"])
                            # prefetch next layer's pass-A masks mid-pass-B
                            if not last and not prefA_done[0] and \
                                    ci >= nchunk // 2:
                                prefetch_masks(f"A{li + 1}", schedA, smTA_in)
                                prefA_done[0] = True
                            for ri in range(len(RS_PLAN)):
                                if posted[ri] or rs_fired_at[ri] is None:
                                    continue
                                ready = (rs_fired_at[ri] + RS_DELAY[ri] <= ci
                                         or ci == nchunk - 1)
                                if ready and (ri == 0 or posted[ri - 1]):
                                    post_range(li, ri, prp, prps)
                                    posted[ri] = True

                        def front_hook_A(ci):
                            if li > 0 and ci == 5:
                                restripe_piece(li, 2)
                            elif li > 0 and ci == 7:
                                restripe_piece(li, 3)

                        edge_pass(schedA, idxA_sb, smTA_in, ag_own,
                                  False, f"A{li}", front_hook=front_hook_A)
                        prefetch_masks(f"B{li}", schedB, smTB_in)
                        edge_pass(schedB, idxB_sb, smTB_in,
                                  table[li % 2], True, f"B{li}", on_back_done)
                        for ri in range(len(RS_PLAN)):
                            if not posted[ri]:
                                post_range(li, ri, prp, prps)
                                posted[ri] = True
    nc.finalize()
    return nc


def _make_consts(W0, al0, ar0, b0, W1, al1, ar1, b1, W2, al2, ar2, b2):
    consts = {}
    for li, (W, al, ar, b) in enumerate(
            [(W0, al0, ar0, b0), (W1, al1, ar1, b1), (W2, al2, ar2, b2)]):
        consts[f"Waug{li}"] = _augment(np.asarray(W, np.float32),
                                       np.asarray(al, np.float32),
                                       np.asarray(ar, np.float32)).astype(
                                           ml_dtypes.bfloat16)
        b = np.asarray(b, np.float32)
        if li < 2:
            consts[f"brep{li}"] = np.tile(b.reshape(1, 128), (128, 1))
        else:
            consts[f"brep{li}"] = np.tile(b.reshape(H, D).mean(0).reshape(1, D),
                                          (128, 1))
    dummy = np.zeros((1, TCOLS), ml_dtypes.bfloat16)
    dummy[0, 128:132] = ml_dtypes.bfloat16(-1e30)
    consts["dummyrow"] = dummy
    return consts


def _in_maps(x):
    cores = _cache["pre"][0]
    x = np.asarray(x, dtype=np.float32)
    in_maps = []
    for c in range(NCORE):
        lo, hi = _own_rows(c)
        xT = np.zeros((128, OWN), ml_dtypes.bfloat16)
        xT[:, 0:NPC] = x[lo:hi].T.astype(ml_dtypes.bfloat16)
        cc = cores[c]
        in_maps.append(dict(xT=xT, idxA=cc["idxA"], idxB=cc["idxB"],
                            smTA=np.asarray(cc["smTA"]),
                            smTB=np.asarray(cc["smTB"])))
    return in_maps


def kernel(x, src, dst, W0, al0, ar0, b0, W1, al1, ar1, b1, W2, al2, ar2, b2):
    from concourse.bass_utils import run_bass_kernel_spmd

    key = (hash(np.asarray(src).tobytes()) ^ hash(np.asarray(dst).tobytes()))
    if "pre" not in _cache or _cache.get("prekey") != key:
        _cache["pre"] = _preprocess(src, dst)
        _cache["prekey"] = key
    cores, schedA, schedB = _cache["pre"]

    consts = _make_consts(W0, al0, ar0, b0, W1, al1, ar1, b1, W2, al2, ar2, b2)

    ck = key ^ hash(consts["Waug0"].tobytes())
    if "nc" not in _cache or _cache.get("nckey") != ck:
        _cache["nc"] = _build(schedA, schedB, consts)
        _cache["nckey"] = ck
    nc = _cache["nc"]

    in_maps = _in_maps(x)
    r = run_bass_kernel_spmd(nc, in_maps, list(range(NCORE)))
    y = np.zeros((N, D), np.float32)
    for c in range(NCORE):
        lo, hi = _own_rows(c)
        y[lo:hi] = r.results[c]["y"]
    return y
